# revision 19
# baseline (speedup 1.0000x reference)
"""Trainium2 Bass kernel for the NRI CNNEncoder (gnn_message_passing).

Strategy
--------
8-way shard over the edge dimension E=1560: each core owns 5 receiver nodes
x 40 sender slots (the self-edge is computed as padding and discarded on the
host) = 200 local edges x B=4 batches = 800 edge-sequences per core.

Algebraic restructuring (all exact, eval-mode):
- conv1 is linear, so per-edge conv1(concat(send, recv)) = F_s[send] + F_r[recv]
  where F_s/F_r are convolutions of the 160 node sequences with the two halves
  of conv1_w: a 39x compute reduction on conv1.
- BatchNorm (eval) = per-channel positive-scale affine; it commutes with
  maxpool and folds into the following conv/matmul weights (bn1 -> conv2,
  bn2 -> convp/conva).
- convp (1x1) commutes with the attention-weighted temporal pooling, so it is
  applied after pooling: 44x less convp compute.
- ELU is stored shifted: elu(t)+1 = min(exp(t), 1+relu(t)) — two parallel
  engine ops + one combine; the "+1" is folded into the next layer's bias on
  the host.
- edge2node is local per receiver shard; the node2edge gather needs one
  per-batch AllGather of the mlp2 outputs (1.25KB). A dummy AllGather at
  kernel start absorbs the ~50us first-use collective setup.

Matmuls run in bf16 (fp32 PSUM accumulation); softmax in fp32.
"""

import os
import sys
import numpy as np

sys.path.insert(0, "/opt/trn_rl_repo")

import ml_dtypes

BF16 = ml_dtypes.bfloat16

# Problem constants (hardcoded; must match the reference).
B, N, T, D, H, O = 4, 40, 100, 4, 128, 2
E = N * (N - 1)          # 1560
BN_EPS = 1e-5
N_CORES = 8
RPC = N // N_CORES       # receivers per core = 5
EL = RPC * N             # local edges per core (incl. self padding) = 200
PT = T - 4               # conv1 output length = 96
PL = PT // 2             # pooled length = 48
CT = PL - 4              # conv2 output length = 44
CK = 20                  # conv1 contraction = D * K = 4*5
FSTRIP = 480             # F matmul strip (5 nodes x 96)
C2EDGES = 10             # edges per conv2/logits strip
C2STRIP = C2EDGES * CT   # 440


def _np_forward(inputs, rel_rec, rel_send, p):
    """Pure-numpy fp32 replica of the reference (fallback for inputs whose
    rel matrices do not have the NRI one-hot structure)."""
    x32 = inputs.astype(np.float32)
    rr = rel_rec.astype(np.float32)
    rs = rel_send.astype(np.float32)
    xf = x32.reshape(B, N, T * D)
    recv = np.einsum("en,bnf->bef", rr, xf).reshape(B * rr.shape[0], T, D)
    send = np.einsum("en,bnf->bef", rs, xf).reshape(B * rs.shape[0], T, D)
    x = np.concatenate([send.transpose(0, 2, 1), recv.transpose(0, 2, 1)], axis=1)

    def conv1d(x, w, b):
        k = w.shape[2]
        t_out = x.shape[2] - k + 1
        y = np.zeros((x.shape[0], w.shape[0], t_out), np.float32)
        for kk in range(k):
            y += np.einsum("oc,nct->not", w[:, :, kk], x[:, :, kk:kk + t_out])
        return y + b[None, :, None]

    def bn(x, g, b, m, v):
        return (x - m[None, :, None]) / np.sqrt(v[None, :, None] + BN_EPS) \
            * g[None, :, None] + b[None, :, None]

    def elu(x):
        return np.where(x > 0, x, np.expm1(x))

    def mlp(x, w1, b1, w2, b2):
        h = elu(x @ w1 + b1)
        return elu(h @ w2 + b2)

    x = bn(np.maximum(conv1d(x, p["conv1_w"], p["conv1_b"]), 0.0),
           p["bn1_g"], p["bn1_b"], p["bn1_m"], p["bn1_v"])
    n_, c_, t_ = x.shape
    x = x.reshape(n_, c_, t_ // 2, 2).max(axis=-1)
    x = bn(np.maximum(conv1d(x, p["conv2_w"], p["conv2_b"]), 0.0),
           p["bn2_g"], p["bn2_b"], p["bn2_m"], p["bn2_v"])
    pred = conv1d(x, p["convp_w"], p["convp_b"])
    a = conv1d(x, p["conva_w"], p["conva_b"])
    a = np.exp(a - a.max(axis=2, keepdims=True))
    a = a / a.sum(axis=2, keepdims=True)
    x = (pred * a).mean(axis=2).reshape(B, -1, H)
    x = mlp(x, p["mlp1_w1"], p["mlp1_b1"], p["mlp1_w2"], p["mlp1_b2"])
    x_skip = x
    inc = np.einsum("en,beh->bnh", rr, x) / N
    x = mlp(inc, p["mlp2_w1"], p["mlp2_b1"], p["mlp2_w2"], p["mlp2_b2"])
    sn = np.einsum("en,bnh->beh", rs, x)
    rc = np.einsum("en,bnh->beh", rr, x)
    x = np.concatenate([sn, rc, x_skip], axis=2)
    x = mlp(x, p["mlp3_w1"], p["mlp3_b1"], p["mlp3_w2"], p["mlp3_b2"])
    return x @ p["fco_w"] + p["fco_b"]


def _nri_structure(rel_rec, rel_send):
    """If (rel_rec, rel_send) are the NRI fully-connected one-hot matrices,
    return edge_of[r][s] -> global edge index; else None."""
    if rel_rec.shape != (E, N) or rel_send.shape != (E, N):
        return None
    rec_i = np.argmax(rel_rec, axis=1)
    snd_i = np.argmax(rel_send, axis=1)
    eye = np.eye(N, dtype=rel_rec.dtype)
    if not (np.array_equal(rel_rec, eye[rec_i]) and
            np.array_equal(rel_send, eye[snd_i])):
        return None
    edge_of = {}
    for e in range(E):
        r, s = int(rec_i[e]), int(snd_i[e])
        if r == s or (r, s) in edge_of:
            return None
        edge_of[(r, s)] = e
    if len(edge_of) != E:
        return None
    return edge_of


_PROGRAM_CACHE = {}
TRACE = False          # test harness sets True to collect NTFF exec time
LAST_RESULT = None     # BassKernelResults of the last run (when TRACE)


def _build_program():
    """Build + compile the SPMD Bass program (cached per process)."""
    if "nc" in _PROGRAM_CACHE:
        return _PROGRAM_CACHE["nc"]

    import concourse.bacc as bacc
    import concourse.tile as tile
    from concourse import mybir
    from contextlib import ExitStack

    f32 = mybir.dt.float32
    f16 = mybir.dt.float16
    bf16 = mybir.dt.bfloat16
    Alu = mybir.AluOpType
    Act = mybir.ActivationFunctionType

    nc = bacc.Bacc("TRN2", target_bir_lowering=False, debug=False,
                   num_devices=N_CORES)

    def din(name, shape, dt=bf16):
        return nc.dram_tensor(name, shape, dt, kind="ExternalInput").ap()

    p1 = din("p1", [B, CK, N * PT])
    p1r = din("p1r", [B, CK, RPC * PT])
    rel_r = din("rel_r", [EL, RPC])
    w1pack = din("w1pack", [CK, 2 * H])
    wpackF = din("wpackF", [H, 5 * H + 1])
    wpackT = din("wpackT", [H, 10 * H + O])
    bpack = din("bpack", [H, 9], f32)
    relT = din("relT", [N, 2 * EL])
    wph = din("wph", [H, H], f16)

    y = nc.dram_tensor("y", [B, O, EL], f32, kind="ExternalOutput").ap()
    # AllGather payload: per-core [RPC, H] bf16 (only q_a = w31a^T @ x2 is
    # gathered; the receiver term stays local).
    cc_in = nc.dram_tensor("cc_in", [B, RPC, H], bf16).ap()
    cc_out = nc.dram_tensor("cc_out", [B, N, H], bf16,
                        addr_space="Shared").ap()
    # Dummy collective to absorb the ~50us first-use setup cost.
    wu_in = nc.dram_tensor("wu_in", [1, 4], bf16).ap()
    wu_out = nc.dram_tensor("wu_out", [N_CORES, 4], bf16,
                            addr_space="Shared").ap()

    with tile.TileContext(nc) as tc:
        with ExitStack() as ctx:
            singles = ctx.enter_context(tc.tile_pool(name="singles", bufs=1))
            work = ctx.enter_context(tc.tile_pool(name="work", bufs=2))
            work3 = ctx.enter_context(tc.tile_pool(name="work3", bufs=3))
            psum = ctx.enter_context(
                tc.tile_pool(name="psum", bufs=2, space="PSUM"))
            psumc = ctx.enter_context(
                tc.tile_pool(name="psumc", bufs=2, space="PSUM"))
            psuml = ctx.enter_context(
                tc.tile_pool(name="psuml", bufs=1, space="PSUM"))
            dpool = ctx.enter_context(
                tc.tile_pool(name="dpool", bufs=2, space="DRAM"))

            def sload(ap_dram, shape, dt=bf16, name=None, eng=None):
                t = singles.tile(shape, dt,
                                 name=name or f"c_{ap_dram.tensor.name}")
                (eng or nc.sync).dma_start(out=t[:], in_=ap_dram)
                return t

            # --- collective warm-up (first thing on the CC queue) --------
            wu_sb = singles.tile([1, 4], bf16, name="wu_sb")
            nc.vector.memset(wu_sb[:], 0.0)
            nc.gpsimd.dma_start(out=wu_in, in_=wu_sb[:])
            nc.gpsimd.collective_compute(
                "AllGather", mybir.AluOpType.bypass,
                replica_groups=[list(range(N_CORES))],
                ins=[wu_in], outs=[wu_out])

            # --- weights / constants into SBUF (packed DMAs) ------------
            w1pack_sb = sload(w1pack, [CK, 2 * H])
            w1s_sb = w1pack_sb[:, 0:H]
            w1r_sb = w1pack_sb[:, H:2 * H]
            wpackF_sb = sload(wpackF, [H, 5 * H + 1])
            w2_sb = [wpackF_sb[:, k * H:(k + 1) * H] for k in range(5)]
            wa_sb = wpackF_sb[:, 5 * H:5 * H + 1]
            wpackT_sb = sload(wpackT, [H, 10 * H + O], eng=nc.scalar)
            (wp_sb, w11_sb, w12_sb, w21_sb, w22_sb, w31a_sb, w31b_sb,
             w31c_sb, w32_sb, ident_sb) = [
                wpackT_sb[:, k * H:(k + 1) * H] for k in range(10)]
            w31ab_sb = wpackT_sb[:, 5 * H:7 * H]
            wfco_sb = wpackT_sb[:, 10 * H:10 * H + O]
            bpack_sb = sload(bpack, [H, 9], f32)
            (b1_sb, b2p_sb, b11_sb, b12_sb, b21_sb, b22_sb, b31_sb,
             b32_sb) = [bpack_sb[:, k:k + 1] for k in range(8)]
            bfco_sb = bpack_sb[0:O, 8:9]
            rel_ra_sb = sload(rel_r[0:120, :], [120, RPC], name="rel_ra",
                              eng=nc.gpsimd)
            rel_rb_sb = sload(rel_r[120:EL, :], [EL - 120, RPC],
                              name="rel_rb", eng=nc.gpsimd)
            relT_sb = sload(relT, [N, 2 * EL], eng=nc.gpsimd)
            rel_sT_sb = relT_sb[:, 0:EL]
            rel_rT_sb = relT_sb[0:RPC, EL:2 * EL]

            wph_sb = sload(wph, [H, H], dt=f16, name="c_wph")

            # --- persistent accumulators ------------------------------------
            V_all = singles.tile([H, B * EL], f16, tag="V_all")
            X1T = singles.tile([H, B * EL], bf16, tag="X1T")

            def elu(ps, bias_sb, out_sb, comb=None, relu_eng=None):
                """out_sb(bf16) = elu(ps + bias) = min(exp(t)-1, relu(t)).
                Exact: for t<=0 exp(t)-1 = elu <= 0 = relu; for t>0
                exp(t)-1 >= t = relu. Overflow-safe (inf loses the min).
                exp and relu are independent -> run on parallel engines."""
                cols = ps.shape[1]
                ex = work.tile([ps.shape[0], cols], f32, tag="elu_ex")
                nc.scalar.activation(ex[:], ps[:], Act.Exp, bias=bias_sb[:])
                rl = work.tile([ps.shape[0], cols], f32, tag="elu_rl")
                if relu_eng is None:
                    nc.scalar.activation(rl[:], ps[:], Act.Relu,
                                         bias=bias_sb[:])
                else:
                    relu_eng.tensor_scalar(
                        out=rl[:], in0=ps[:], scalar1=bias_sb[:],
                        scalar2=0.0, op0=Alu.add, op1=Alu.max)
                (comb or nc.vector).scalar_tensor_tensor(
                    out=out_sb, in0=ex[:], scalar=-1.0, in1=rl[:],
                    op0=Alu.add, op1=Alu.min)

            # ================= per-batch edge pipeline =======================
            def a_front(b, rr, Fs, Fr):
                """G, fused pool+relu, conv2 matmuls."""
                G = work.tile([H, N * PT], bf16, tag="G", name="G")
                fr_b = Fr[:, rr * PT:(rr + 1) * PT] \
                    .unsqueeze(1).broadcast_to([H, N, PT])
                nc.vector.tensor_tensor(
                    out=G[:].rearrange("p (n t) -> p n t", t=PT),
                    in0=Fs[:].rearrange("p (n t) -> p n t", t=PT),
                    in1=fr_b, op=Alu.add)
                # fused maxpool(k=2) + relu on DVE: max(G_even, G_odd, 0).
                # The host de-interleaves conv1's time columns (even block
                # then odd block per node), so both pool inputs and the
                # output are packed contiguous runs -> DVE 2x mode.
                Y1 = work.tile([H, N * PL], bf16, tag="Y1", name="Y1")
                G3 = G[:].rearrange("p (n x) -> p n x", x=PT)
                nc.vector.scalar_tensor_tensor(
                    out=Y1[:].rearrange("p (e t) -> p e t", t=PL),
                    in0=G3[:, :, 0:PL],
                    scalar=0.0, in1=G3[:, :, PL:PT],
                    op0=Alu.max, op1=Alu.max)
                Y1r = Y1[:].rearrange("p (e t) -> p e t", t=PL)
                # conv2 into two 2-bank PSUM tiles (strips padded to 512
                # cols) so the relu drain needs 2 instructions, not 4.
                c2ps = []
                for h in range(2):
                    ps = psumc.tile([H, 1024], f32, tag="c2",
                                    name=f"c2ps{h}")
                    for sl in range(2):
                        st = 2 * h + sl
                        for k in range(5):
                            nc.tensor.matmul(
                                ps[:, sl * 512:sl * 512 + C2STRIP],
                                lhsT=w2_sb[k][:],
                                rhs=Y1r[:, st * C2EDGES:(st + 1) * C2EDGES,
                                        k:k + CT],
                                start=(k == 0), stop=(k == 4))
                    c2ps.append(ps)
                return (c2ps,)

            def a_tail(b, rr, c2ps):
                """relu2 (PSUM drain), logits, direct PSUM->A_t DMAs."""
                Y = work.tile([H, N * CT], bf16, tag="Y", name="Y", bufs=5)
                for h in range(2):
                    nc.scalar.activation(
                        Y[:, h * 2 * C2STRIP:(h + 1) * 2 * C2STRIP]
                        .rearrange("p (s x) -> p s x", x=C2STRIP),
                        c2ps[h][:].rearrange("p (s x) -> p s x",
                                             x=512)[:, :, 0:C2STRIP],
                        Act.Relu, bias=b2p_sb[:])
                A_t = work.tile([N, CT], f32, tag="A_t", name="A_t", bufs=3)
                Lsb = work.tile([1, N * CT], f32, tag="Lsb", name="Lsb")
                # Two logit strips per 2-bank PSUM tile; each pair drained
                # by one copy (alternating Scalar/Pool to balance queues).
                for h2 in range(2):
                    lps = psuml.tile([1, 1024], f32, tag="lp", name="lps")
                    for sl in range(2):
                        st = 2 * h2 + sl
                        nc.tensor.matmul(
                            lps[:, sl * 512:sl * 512 + C2STRIP],
                            lhsT=wa_sb[:],
                            rhs=Y[:, st * C2STRIP:(st + 1) * C2STRIP],
                            start=True, stop=True)
                    nc.scalar.copy(
                        Lsb[:, h2 * 2 * C2STRIP:(h2 + 1) * 2 * C2STRIP]
                        .rearrange("p (s x) -> p s x", x=C2STRIP),
                        lps[:].rearrange("p (s x) -> p s x",
                                         x=512)[:, :, 0:C2STRIP])
                nc.sync.dma_start(out=A_t[:], in_=Lsb[:])
                return Y, A_t

            def sm_stage(b, rr, Y, A_t):
                """Softmax + partition-broadcast. No max-subtraction: the
                attention logits here are O(1), far from fp32 exp range."""
                Ex = work.tile([N, CT], f32, tag="Ex", name="Ex")
                S = work.tile([N, 1], f32, tag="S", name="S")
                nc.scalar.activation(Ex[:], A_t[:], Act.Exp,
                                     accum_out=S[:])
                rz = work.tile([N, 1], f32, tag="rz", name="rz")
                nc.vector.reciprocal(rz[:], S[:])
                A_bf = work.tile([N, CT], bf16, tag="A_bf", name="A_bf")
                nc.vector.tensor_scalar(
                    out=A_bf[:], in0=Ex[:], scalar1=rz[:],
                    scalar2=1.0 / CT, op0=Alu.mult, op1=Alu.mult)
                # Bounce through DRAM to broadcast across partitions. The
                # write and the broadcast-read are issued on different
                # queues so independent per-block streams don't block each
                # other head-of-line.
                A_dram = dpool.tile([1, N * CT], bf16, tag="A_dram",
                                    name="A_dram", bufs=4)
                nc.gpsimd.dma_start(out=A_dram[:], in_=A_bf[:])
                A_bc = work.tile([H, N * CT], bf16, tag="A_bc", name="A_bc",
                                 bufs=5)
                nc.sync.dma_start(
                    out=A_bc[:],
                    in_=A_dram[0:1, :].broadcast_to([H, N * CT]))
                return b, rr, Y, A_bc

            def mt_mult(b, rr, Y, A_bc):
                """Weighted temporal multiply. Runs on the Pool engine for
                some blocks to relieve the DVE; the segmented reduce is
                issued one iteration later (mt_reduce) so the DVE FIFO
                never stalls waiting for the slower Pool multiply."""
                eng = nc.gpsimd if rr in (1, 3) else nc.vector
                Mt = work.tile([H, N * CT], f16, tag="Mt", name="Mt",
                               bufs=3)
                eng.tensor_tensor(out=Mt[:], in0=Y[:], in1=A_bc[:],
                                  op=Alu.mult)
                return b, rr, Mt

            def mt_reduce(b, rr, Mt):
                col0 = (b * RPC + rr) * N
                with nc.allow_low_precision(
                        reason="fp16 attention-pool accum, |terms|<=44"):
                    nc.vector.tensor_reduce(
                        out=V_all[:, col0:col0 + N],
                        in_=Mt[:].rearrange("p (e t) -> p e t", t=CT),
                        axis=mybir.AxisListType.X, op=Alu.add)

            def phase_b_part(b, c0, cw, comb=None):
                """convp (folded post-pooling) + mlp1 for cols [c0, c0+cw)
                of batch b."""
                cs = slice(b * EL + c0, b * EL + c0 + cw)
                zps = psum.tile([H, cw], f32, tag="ps", name="zps")
                nc.tensor.matmul(zps[:], lhsT=wph_sb[:], rhs=V_all[:, cs],
                                 start=True, stop=True)
                xsb = work.tile([H, cw], bf16, tag="xsb", name="xsb")
                nc.scalar.copy(xsb[:], zps[:])
                h1ps = psum.tile([H, cw], f32, tag="ps", name="h1ps")
                nc.tensor.matmul(h1ps[:], lhsT=w11_sb[:], rhs=xsb[:],
                                 start=True, stop=True)
                h1sb = work.tile([H, cw], bf16, tag="h1sb", name="h1sb")
                elu(h1ps, b11_sb, h1sb[:], comb=comb)
                h2ps = psum.tile([H, cw], f32, tag="ps", name="h2ps")
                nc.tensor.matmul(h2ps[:], lhsT=w12_sb[:], rhs=h1sb[:],
                                 start=True, stop=True)
                elu(h2ps, b12_sb, X1T[:, cs], comb=comb)

            h3part = singles.tile([H, 2 * EL], bf16, tag="h3part")

            INC_CHUNKS = [(0, 120), (120, EL - 120)]

            def inc_tr(b, j, incps):
                """Transpose one X1T chunk of batch b and accumulate its
                edge2node partial into incps."""
                c0, cw = INC_CHUNKS[j]
                tps = psum.tile([cw, H], bf16, tag="ps", name="tps")
                nc.tensor.transpose(
                    tps[:], in_=X1T[:, b * EL + c0:b * EL + c0 + cw],
                    identity=ident_sb[:])
                x1e = work3.tile([cw, H], bf16, tag=f"x1e{j}", name="x1e")
                nc.scalar.copy(x1e[:], tps[:])
                rel_chunk = rel_ra_sb if j == 0 else rel_rb_sb
                nc.tensor.matmul(
                    incps[:], lhsT=x1e[:],
                    rhs=rel_chunk[:], start=(j == 0), stop=(j == 1))

            def inc_fin(b, incps):
                """mlp2 on the local RPC nodes + mlp3 weight folding ->
                [RPC, H] gather payload."""
                inc_sb = work3.tile([H, RPC], bf16, tag="inc_sb",
                                    name="inc_sb")
                nc.scalar.copy(inc_sb[:], incps[:])
                m2ps = psum.tile([H, RPC], f32, tag="ps", name="m2ps")
                nc.tensor.matmul(m2ps[:], lhsT=w21_sb[:], rhs=inc_sb[:],
                                 start=True, stop=True)
                m2sb = work3.tile([H, RPC], bf16, tag="m2sb", name="m2sb")
                elu(m2ps, b21_sb, m2sb[:], relu_eng=nc.vector)
                m2ps2 = psum.tile([H, RPC], f32, tag="ps", name="m2ps2")
                nc.tensor.matmul(m2ps2[:], lhsT=w22_sb[:], rhs=m2sb[:],
                                 start=True, stop=True)
                x2t = work3.tile([H, RPC], bf16, tag="x2t", name="x2t")
                elu(m2ps2, b22_sb, x2t[:], relu_eng=nc.vector)
                # qaT = x2^T @ w31a (sender term -> gathered),
                # qbT = x2^T @ w31b (receiver term -> stays local);
                # one merged matmul [RPC, 2H]; lhsT = x2 keeps the gather
                # payload and the rc-gather lhsT at partition 0.
                qps = psum.tile([RPC, 2 * H], f32, tag="ps", name="qps")
                nc.tensor.matmul(qps[:], lhsT=x2t[:], rhs=w31ab_sb[:],
                                 start=True, stop=True)
                qt_sb = work3.tile([RPC, 2 * H], bf16, tag="qt_sb",
                                   name="qt_sb")
                nc.scalar.copy(qt_sb[:], qps[:])
                nc.gpsimd.dma_start(out=cc_in[b], in_=qt_sb[:, 0:H])
                return qt_sb

            def cc_b(b):
                nc.gpsimd.collective_compute(
                    "AllGather", mybir.AluOpType.bypass,
                    replica_groups=[list(range(N_CORES))],
                    ins=[cc_in[b]], outs=[cc_out[b]])

            def h3_pre(b, qt_sb):
                """Pre-collective part of mlp3: receiver + skip terms."""
                h3p = psum.tile([H, EL], f32, tag="ps", name="h3p")
                nc.tensor.matmul(h3p[:], lhsT=qt_sb[:, H:2 * H],
                                 rhs=rel_rT_sb[:], start=True, stop=False)
                nc.tensor.matmul(h3p[:], lhsT=w31c_sb[:],
                                 rhs=X1T[:, b * EL:(b + 1) * EL],
                                 start=False, stop=True)
                nc.scalar.copy(h3part[:, (b % 2) * EL:(b % 2 + 1) * EL],
                               h3p[:])

            def f_dma(b):
                """Prefetch the im2col inputs for batch b. All four batches
                are issued upfront on the gpsimd queue (cheap issue, no
                data-dependent traffic ahead of them)."""
                p1_sb = work.tile([CK, N * PT], bf16, tag="p1_sb",
                                  name="p1_sb", bufs=4)
                nc.gpsimd.dma_start(out=p1_sb[:], in_=p1[b])
                p1r_sb = work.tile([CK, RPC * PT], bf16, tag="p1r_sb",
                                   name="p1r_sb", bufs=4)
                nc.gpsimd.dma_start(out=p1r_sb[:], in_=p1r[b])
                return p1_sb, p1r_sb

            def f_stage(b, p1_sb, p1r_sb):
                Fs = work.tile([H, N * PT], bf16, tag="Fs", name="Fs")
                for s4 in range(N * PT // (2 * FSTRIP)):
                    fps = psumc.tile([H, 1024], f32, tag="c2", name="fps")
                    for sl in range(2):
                        s8 = 2 * s4 + sl
                        nc.tensor.matmul(
                            fps[:, sl * 512:sl * 512 + FSTRIP],
                            lhsT=w1s_sb[:],
                            rhs=p1_sb[:, s8 * FSTRIP:(s8 + 1) * FSTRIP],
                            start=True, stop=True)
                    nc.scalar.copy(
                        Fs[:, s4 * 2 * FSTRIP:(s4 + 1) * 2 * FSTRIP]
                        .rearrange("p (s x) -> p s x", x=FSTRIP),
                        fps[:].rearrange("p (s x) -> p s x",
                                         x=512)[:, :, 0:FSTRIP])
                Fr = work.tile([H, RPC * PT], bf16, tag="Fr", name="Fr")
                frps = psum.tile([H, RPC * PT], f32, tag="ps", name="frps")
                nc.tensor.matmul(frps[:], lhsT=w1r_sb[:], rhs=p1r_sb[:],
                                 start=True, stop=True)
                nc.scalar.activation(Fr[:], frps[:], Act.Identity,
                                     bias=b1_sb[:])
                return Fs, Fr

            def tail_stage(b):
                """Post-collective: sender gather + mlp3 + fco + output."""
                qg_sb = work.tile([N, H], bf16, tag="qg", name="qg_sb")
                nc.scalar.dma_start(out=qg_sb[:], in_=cc_out[b])
                h3ps = psum.tile([H, EL], f32, tag="ps", name="h3ps")
                nc.tensor.matmul(h3ps[:], lhsT=qg_sb[:], rhs=rel_sT_sb[:],
                                 start=True, stop=False)
                nc.tensor.matmul(h3ps[:], lhsT=ident_sb[:],
                                 rhs=h3part[:, (b % 2) * EL:(b % 2 + 1) * EL],
                                 start=False, stop=True)
                h3sb = work.tile([H, EL], bf16, tag="h3sb", name="h3sb")
                elu(h3ps, b31_sb, h3sb[:], relu_eng=nc.vector)
                h4ps = psum.tile([H, EL], f32, tag="ps", name="h4ps")
                nc.tensor.matmul(h4ps[:], lhsT=w32_sb[:], rhs=h3sb[:],
                                 start=True, stop=True)
                h4sb = work.tile([H, EL], bf16, tag="h4sb", name="h4sb")
                elu(h4ps, b32_sb, h4sb[:], relu_eng=nc.vector)
                ops = psum.tile([O, EL], f32, tag="ps", name="ops")
                nc.tensor.matmul(ops[:], lhsT=wfco_sb[:], rhs=h4sb[:],
                                 start=True, stop=True)
                osb = work.tile([O, EL], f32, tag="osb", name="osb")
                nc.vector.tensor_scalar_add(osb[:], ops[:], bfco_sb[:])
                nc.scalar.dma_start(out=y[b], in_=osb[:])

            # Software pipeline over edge-blocks:
            #   a_front(i) -> sm(i-1) -> mt(i-2)+convp/mlp1 chunk -> a_tail(i)
            # with per-batch edge2node/mlp2/AllGather and the post-collective
            # tail overlapped under later blocks.
            # Software pipeline (block i processed at iteration i):
            #   a_tail(i) -> sm(i) at iter i+1 -> Mt multiply at iter i+3
            #   -> segmented reduce + convp/mlp1 chunk at iter i+4.
            # The deep lag hides the long cross-engine softmax/broadcast
            # chain; the reduce lags the multiply so the DVE FIFO never
            # waits on the Pool engine.
            from collections import deque
            pend_sm = None
            pend_mults = deque()
            pend_red = None
            FsFr = None
            qts = {}
            p1s = [f_dma(b) for b in range(B)]
            blocks = [(b, rr) for b in range(B) for rr in range(RPC)]

            def do_reduce_and_part(red):
                rb, rrr, Mt = red
                mt_reduce(rb, rrr, Mt)
                phase_b_part(rb, rrr * N, N)

            def batch_tail(bb):
                """Strip-level edge2node -> mlp2 -> AllGather issue."""
                incps = psum.tile([H, RPC], f32, tag="ps", name="incps")
                inc_tr(bb, 0, incps)
                inc_tr(bb, 1, incps)
                qts[bb] = inc_fin(bb, incps)
                cc_b(bb)

            for i, (b, rr) in enumerate(blocks):
                if rr == 0:
                    FsFr = f_stage(b, *p1s[b])
                fr = a_front(b, rr, *FsFr)
                if pend_red is not None:
                    do_reduce_and_part(pend_red)
                    pend_red = None
                if len(pend_mults) >= 2:
                    pend_red = mt_mult(*pend_mults.popleft())
                if pend_sm is not None:
                    pend_mults.append(sm_stage(*pend_sm))
                tiles = a_tail(b, rr, *fr)
                pend_sm = (b, rr) + tiles
                if i in (8, 13, 18):
                    batch_tail((i - 8) // 5)
                if i in (9, 14, 19):
                    h3_pre((i - 9) // 5, qts[(i - 9) // 5])
                if i in (12, 17):
                    tail_stage((i - 12) // 5)
            # Endgame: drain the pipeline for batch 3, then its serial
            # tail: edge2node -> mlp2 -> AllGather -> mlp3 -> out.
            m32 = mt_mult(*pend_mults.popleft())                 # (3,2)
            sm_out = sm_stage(*pend_sm)                          # (3,4) sm
            do_reduce_and_part(pend_red)                         # (3,1)
            m33 = mt_mult(*pend_mults.popleft())                 # (3,3)
            tail_stage(2)
            do_reduce_and_part(m32)
            m34 = mt_mult(*sm_out)                               # (3,4)
            do_reduce_and_part(m33)
            do_reduce_and_part(m34)
            incps3 = psum.tile([H, RPC], f32, tag="ps", name="incps")
            inc_tr(3, 0, incps3)
            inc_tr(3, 1, incps3)
            qt = inc_fin(3, incps3)
            cc_b(3)
            h3_pre(3, qt)
            tail_stage(3)

    nc.compile()
    _PROGRAM_CACHE["nc"] = nc
    return nc


def _host_prep(inputs, rel_rec, rel_send, p, edge_of):
    """Build the per-core input maps + (core, local, global) output mapping."""
    x = inputs.astype(np.float32)
    # im2col of the node time-series: P1[b, c*5+k, n*96+t] = x[b, n, t+k, c]
    win = np.lib.stride_tricks.sliding_window_view(x, 5, axis=2)  # [B,N,96,D,5]
    P1 = win.transpose(0, 3, 4, 1, 2).reshape(B, CK, N, PT)
    # De-interleave each node's conv1 time columns (even positions first,
    # then odd) so the kernel's fused maxpool reads packed contiguous runs
    # (DVE 2x mode) instead of stride-2 views.
    perm = np.concatenate([np.arange(0, PT, 2), np.arange(1, PT, 2)])
    P1 = np.ascontiguousarray(P1[:, :, :, perm])

    a1 = (p["bn1_g"] / np.sqrt(p["bn1_v"] + BN_EPS)).astype(np.float32)
    c1 = (p["bn1_b"] - p["bn1_m"] * a1).astype(np.float32)
    a2 = (p["bn2_g"] / np.sqrt(p["bn2_v"] + BN_EPS)).astype(np.float32)
    c2 = (p["bn2_b"] - p["bn2_m"] * a2).astype(np.float32)

    w1 = p["conv1_w"].astype(np.float32)           # [H, 2D, 5]
    # rows ordered c*5+k to match P1
    W1s = w1[:, :D, :].transpose(1, 2, 0).reshape(CK, H)
    W1r = w1[:, D:, :].transpose(1, 2, 0).reshape(CK, H)

    w2f = p["conv2_w"].astype(np.float32) * a1[None, :, None]   # [o,i,k]
    b2p = p["conv2_b"].astype(np.float32) + np.einsum(
        "oik,i->o", p["conv2_w"].astype(np.float32), c1)
    W2k = [w2f[:, :, k].T.copy() for k in range(5)]             # lhsT [i,o]

    wa = (p["conva_w"][0, :, 0].astype(np.float32) * a2)[:, None]  # [H,1]
    WpT = (p["convp_w"][:, :, 0].astype(np.float32) * a2[None, :]).T  # [i,o]
    bpp = p["convp_b"].astype(np.float32) + \
        p["convp_w"][:, :, 0].astype(np.float32) @ c2

    m1w1 = p["mlp1_w1"].astype(np.float32)
    m1w2 = p["mlp1_w2"].astype(np.float32)
    b11 = p["mlp1_b1"].astype(np.float32) + (bpp / CT) @ m1w1
    b12 = p["mlp1_b2"].astype(np.float32)
    m2w1 = p["mlp2_w1"].astype(np.float32)
    m2w2 = p["mlp2_w2"].astype(np.float32)
    W21 = m2w1 / N
    b21 = p["mlp2_b1"].astype(np.float32)
    b22 = p["mlp2_b2"].astype(np.float32)
    m3w1 = p["mlp3_w1"].astype(np.float32)
    m3w2 = p["mlp3_w2"].astype(np.float32)
    b31 = p["mlp3_b1"].astype(np.float32)
    b32 = p["mlp3_b2"].astype(np.float32)
    bfco = p["fco_b"].astype(np.float32)

    wpackT = np.concatenate(
        [WpT, m1w1, m1w2, W21, m2w2, m3w1[0:H],
         m3w1[H:2 * H], m3w1[2 * H:3 * H], m3w2,
         np.eye(H, dtype=np.float32), p["fco_w"]],
        axis=1).astype(BF16)
    bpack = np.zeros((H, 9), np.float32)
    for k, v in enumerate([p["conv1_b"], b2p, b11, b12, b21, b22, b31, b32]):
        bpack[:, k] = v
    bpack[:O, 8] = bfco
    shared = {
        "p1": P1.reshape(B, CK, N * PT).astype(BF16),
        "w1pack": np.concatenate([W1s, W1r], axis=1).astype(BF16),
        "wpackF": np.concatenate(W2k + [wa], axis=1).astype(BF16),
        "wpackT": wpackT,
        "bpack": bpack,
        "wph": WpT.astype(np.float16),
    }

    in_maps = []
    out_map = []  # (core, e_loc, e_glob)
    for c in range(N_CORES):
        recvs = list(range(c * RPC, (c + 1) * RPC))
        relr = np.zeros((EL, RPC), np.float32)
        relsT = np.zeros((N, EL), np.float32)
        relrT = np.zeros((N, EL), np.float32)
        for rr_i, r in enumerate(recvs):
            for s in range(N):
                if s == r:
                    continue
                e_loc = rr_i * N + s
                e_g = edge_of[(r, s)]
                relr[e_loc, rr_i] = 1.0
                relsT[s, e_loc] = 1.0
                relrT[rr_i, e_loc] = 1.0
                out_map.append((c, e_loc, e_g))
        m = dict(shared)
        m["p1r"] = np.ascontiguousarray(
            P1[:, :, recvs, :]).reshape(B, CK, RPC * PT).astype(BF16)
        m["rel_r"] = relr.astype(BF16)
        m["relT"] = np.concatenate([relsT, relrT], axis=1).astype(BF16)
        in_maps.append(m)
    return in_maps, out_map


def kernel(**inputs):
    rel_rec = np.asarray(inputs["rel_rec"])
    rel_send = np.asarray(inputs["rel_send"])
    x = np.asarray(inputs["inputs"])
    p = {k: np.asarray(v) for k, v in inputs.items()
         if k not in ("inputs", "rel_rec", "rel_send")}

    edge_of = _nri_structure(rel_rec, rel_send)
    if edge_of is None or x.shape != (B, N, T, D):
        # Inputs without the NRI one-hot structure: fall back to a plain
        # numpy evaluation (correctness path only).
        return _np_forward(x, rel_rec, rel_send, p).astype(np.float32)

    from concourse.bass_utils import run_bass_kernel_spmd

    nc = _build_program()
    in_maps, out_map = _host_prep(x, rel_rec, rel_send, p, edge_of)
    res = run_bass_kernel_spmd(nc, in_maps, list(range(N_CORES)),
                               trace=TRACE)
    if TRACE:
        global LAST_RESULT
        LAST_RESULT = res

    full = np.empty((B, E, O), np.float32)
    for c, e_loc, e_g in out_map:
        full[:, e_g, :] = res.results[c]["y"][:, :, e_loc]
    return full


# revision 25
# speedup vs baseline: 1.0328x; 1.0328x over previous
"""Trainium2 Bass kernel for the NRI CNNEncoder (gnn_message_passing).

Strategy
--------
8-way shard over the edge dimension E=1560: each core owns 5 receiver nodes
x 40 sender slots (the self-edge is computed as padding and discarded on the
host) = 200 local edges x B=4 batches = 800 edge-sequences per core.

Algebraic restructuring (all exact, eval-mode):
- conv1 is linear, so per-edge conv1(concat(send, recv)) = F_s[send] + F_r[recv]
  where F_s/F_r are convolutions of the 160 node sequences with the two halves
  of conv1_w: a 39x compute reduction on conv1.
- BatchNorm (eval) = per-channel positive-scale affine; it commutes with
  maxpool and folds into the following conv/matmul weights (bn1 -> conv2,
  bn2 -> convp/conva).
- convp (1x1) commutes with the attention-weighted temporal pooling, so it is
  applied after pooling: 44x less convp compute.
- ELU is stored shifted: elu(t)+1 = min(exp(t), 1+relu(t)) — two parallel
  engine ops + one combine; the "+1" is folded into the next layer's bias on
  the host.
- edge2node is local per receiver shard; the node2edge gather needs one
  per-batch AllGather of the mlp2 outputs (1.25KB). A dummy AllGather at
  kernel start absorbs the ~50us first-use collective setup.

Matmuls run in bf16 (fp32 PSUM accumulation); softmax in fp32.
"""

import os
import sys
import numpy as np

sys.path.insert(0, "/opt/trn_rl_repo")

import ml_dtypes

BF16 = ml_dtypes.bfloat16

# Problem constants (hardcoded; must match the reference).
B, N, T, D, H, O = 4, 40, 100, 4, 128, 2
E = N * (N - 1)          # 1560
BN_EPS = 1e-5
N_CORES = 8
RPC = N // N_CORES       # receivers per core = 5
EL = RPC * N             # local edges per core (incl. self padding) = 200
PT = T - 4               # conv1 output length = 96
PL = PT // 2             # pooled length = 48
CT = PL - 4              # conv2 output length = 44
CK = 20                  # conv1 contraction = D * K = 4*5
FSTRIP = 480             # F matmul strip (5 nodes x 96)
C2EDGES = 10             # edges per conv2/logits strip
C2STRIP = C2EDGES * CT   # 440


def _np_forward(inputs, rel_rec, rel_send, p):
    """Pure-numpy fp32 replica of the reference (fallback for inputs whose
    rel matrices do not have the NRI one-hot structure)."""
    x32 = inputs.astype(np.float32)
    rr = rel_rec.astype(np.float32)
    rs = rel_send.astype(np.float32)
    xf = x32.reshape(B, N, T * D)
    recv = np.einsum("en,bnf->bef", rr, xf).reshape(B * rr.shape[0], T, D)
    send = np.einsum("en,bnf->bef", rs, xf).reshape(B * rs.shape[0], T, D)
    x = np.concatenate([send.transpose(0, 2, 1), recv.transpose(0, 2, 1)], axis=1)

    def conv1d(x, w, b):
        k = w.shape[2]
        t_out = x.shape[2] - k + 1
        y = np.zeros((x.shape[0], w.shape[0], t_out), np.float32)
        for kk in range(k):
            y += np.einsum("oc,nct->not", w[:, :, kk], x[:, :, kk:kk + t_out])
        return y + b[None, :, None]

    def bn(x, g, b, m, v):
        return (x - m[None, :, None]) / np.sqrt(v[None, :, None] + BN_EPS) \
            * g[None, :, None] + b[None, :, None]

    def elu(x):
        return np.where(x > 0, x, np.expm1(x))

    def mlp(x, w1, b1, w2, b2):
        h = elu(x @ w1 + b1)
        return elu(h @ w2 + b2)

    x = bn(np.maximum(conv1d(x, p["conv1_w"], p["conv1_b"]), 0.0),
           p["bn1_g"], p["bn1_b"], p["bn1_m"], p["bn1_v"])
    n_, c_, t_ = x.shape
    x = x.reshape(n_, c_, t_ // 2, 2).max(axis=-1)
    x = bn(np.maximum(conv1d(x, p["conv2_w"], p["conv2_b"]), 0.0),
           p["bn2_g"], p["bn2_b"], p["bn2_m"], p["bn2_v"])
    pred = conv1d(x, p["convp_w"], p["convp_b"])
    a = conv1d(x, p["conva_w"], p["conva_b"])
    a = np.exp(a - a.max(axis=2, keepdims=True))
    a = a / a.sum(axis=2, keepdims=True)
    x = (pred * a).mean(axis=2).reshape(B, -1, H)
    x = mlp(x, p["mlp1_w1"], p["mlp1_b1"], p["mlp1_w2"], p["mlp1_b2"])
    x_skip = x
    inc = np.einsum("en,beh->bnh", rr, x) / N
    x = mlp(inc, p["mlp2_w1"], p["mlp2_b1"], p["mlp2_w2"], p["mlp2_b2"])
    sn = np.einsum("en,bnh->beh", rs, x)
    rc = np.einsum("en,bnh->beh", rr, x)
    x = np.concatenate([sn, rc, x_skip], axis=2)
    x = mlp(x, p["mlp3_w1"], p["mlp3_b1"], p["mlp3_w2"], p["mlp3_b2"])
    return x @ p["fco_w"] + p["fco_b"]


def _nri_structure(rel_rec, rel_send):
    """If (rel_rec, rel_send) are the NRI fully-connected one-hot matrices,
    return edge_of[r][s] -> global edge index; else None."""
    if rel_rec.shape != (E, N) or rel_send.shape != (E, N):
        return None
    rec_i = np.argmax(rel_rec, axis=1)
    snd_i = np.argmax(rel_send, axis=1)
    eye = np.eye(N, dtype=rel_rec.dtype)
    if not (np.array_equal(rel_rec, eye[rec_i]) and
            np.array_equal(rel_send, eye[snd_i])):
        return None
    edge_of = {}
    for e in range(E):
        r, s = int(rec_i[e]), int(snd_i[e])
        if r == s or (r, s) in edge_of:
            return None
        edge_of[(r, s)] = e
    if len(edge_of) != E:
        return None
    return edge_of


_PROGRAM_CACHE = {}
TRACE = False          # test harness sets True to collect NTFF exec time
LAST_RESULT = None     # BassKernelResults of the last run (when TRACE)


def _build_program():
    """Build + compile the SPMD Bass program (cached per process)."""
    if "nc" in _PROGRAM_CACHE:
        return _PROGRAM_CACHE["nc"]

    import concourse.bacc as bacc
    import concourse.tile as tile
    from concourse import mybir
    from contextlib import ExitStack

    f32 = mybir.dt.float32
    f16 = mybir.dt.float16
    bf16 = mybir.dt.bfloat16
    Alu = mybir.AluOpType
    Act = mybir.ActivationFunctionType

    nc = bacc.Bacc("TRN2", target_bir_lowering=False, debug=False,
                   num_devices=N_CORES)

    def din(name, shape, dt=bf16):
        return nc.dram_tensor(name, shape, dt, kind="ExternalInput").ap()

    p1 = din("p1", [B, CK, N * PT])
    p1r = din("p1r", [B, CK, RPC * PT])
    rel_r = din("rel_r", [EL, RPC])
    w1pack = din("w1pack", [CK, 2 * H])
    wpackF = din("wpackF", [H, 5 * H + 1])
    wpackT = din("wpackT", [H, 10 * H + O])
    bpack = din("bpack", [H, 9], f32)
    relT = din("relT", [N, 2 * EL])
    wph = din("wph", [H, H], f16)

    y = nc.dram_tensor("y", [B, O, EL], f32, kind="ExternalOutput").ap()
    # AllGather payload: per-core [RPC, H] bf16 (only q_a = w31a^T @ x2 is
    # gathered; the receiver term stays local).
    cc_in = nc.dram_tensor("cc_in", [B, RPC, H], bf16).ap()
    cc_out = nc.dram_tensor("cc_out", [B, N, H], bf16,
                        addr_space="Shared").ap()
    # Dummy collective to absorb the ~50us first-use setup cost.
    wu_in = nc.dram_tensor("wu_in", [1, 4], bf16).ap()
    wu_out = nc.dram_tensor("wu_out", [N_CORES, 4], bf16,
                            addr_space="Shared").ap()

    with tile.TileContext(nc) as tc:
        with ExitStack() as ctx:
            singles = ctx.enter_context(tc.tile_pool(name="singles", bufs=1))
            work = ctx.enter_context(tc.tile_pool(name="work", bufs=2))
            work3 = ctx.enter_context(tc.tile_pool(name="work3", bufs=3))
            psum = ctx.enter_context(
                tc.tile_pool(name="psum", bufs=2, space="PSUM"))
            psumc = ctx.enter_context(
                tc.tile_pool(name="psumc", bufs=2, space="PSUM"))
            psuml = ctx.enter_context(
                tc.tile_pool(name="psuml", bufs=1, space="PSUM"))
            dpool = ctx.enter_context(
                tc.tile_pool(name="dpool", bufs=2, space="DRAM"))

            def sload(ap_dram, shape, dt=bf16, name=None, eng=None):
                t = singles.tile(shape, dt,
                                 name=name or f"c_{ap_dram.tensor.name}")
                (eng or nc.sync).dma_start(out=t[:], in_=ap_dram)
                return t

            # --- collective warm-up (first thing on the CC queue) --------
            wu_sb = singles.tile([1, 4], bf16, name="wu_sb")
            nc.vector.memset(wu_sb[:], 0.0)
            nc.gpsimd.dma_start(out=wu_in, in_=wu_sb[:])
            nc.gpsimd.collective_compute(
                "AllGather", mybir.AluOpType.bypass,
                replica_groups=[list(range(N_CORES))],
                ins=[wu_in], outs=[wu_out])

            # --- weights / constants into SBUF (packed DMAs) ------------
            # w1pack + batch 0's im2col inputs first: the first f_stage
            # needs exactly these, everything else can trickle in after.
            w1pack_sb = sload(w1pack, [CK, 2 * H])
            p1s0 = work.tile([CK, N * PT], bf16, tag="p1_sb",
                             name="p1_sb", bufs=4)
            nc.sync.dma_start(out=p1s0[:], in_=p1[0])
            p1r_s0 = work.tile([CK, RPC * PT], bf16, tag="p1r_sb",
                               name="p1r_sb", bufs=4)
            nc.sync.dma_start(out=p1r_s0[:], in_=p1r[0])
            w1s_sb = w1pack_sb[:, 0:H]
            w1r_sb = w1pack_sb[:, H:2 * H]
            wpackF_sb = sload(wpackF, [H, 5 * H + 1])
            w2_sb = [wpackF_sb[:, k * H:(k + 1) * H] for k in range(5)]
            wa_sb = wpackF_sb[:, 5 * H:5 * H + 1]
            wpackT_sb = sload(wpackT, [H, 10 * H + O], eng=nc.scalar)
            (wp_sb, w11_sb, w12_sb, w21_sb, w22_sb, w31a_sb, w31b_sb,
             w31c_sb, w32_sb, ident_sb) = [
                wpackT_sb[:, k * H:(k + 1) * H] for k in range(10)]
            w31ab_sb = wpackT_sb[:, 5 * H:7 * H]
            wfco_sb = wpackT_sb[:, 10 * H:10 * H + O]
            bpack_sb = sload(bpack, [H, 9], f32)
            (b1_sb, b2p_sb, b11_sb, b12_sb, b21_sb, b22_sb, b31_sb,
             b32_sb) = [bpack_sb[:, k:k + 1] for k in range(8)]
            bfco_sb = bpack_sb[0:O, 8:9]
            rel_ra_sb = sload(rel_r[0:120, :], [120, RPC], name="rel_ra",
                              eng=nc.gpsimd)
            rel_rb_sb = sload(rel_r[120:EL, :], [EL - 120, RPC],
                              name="rel_rb", eng=nc.gpsimd)
            relT_sb = sload(relT, [N, 2 * EL], eng=nc.gpsimd)
            rel_sT_sb = relT_sb[:, 0:EL]
            rel_rT_sb = relT_sb[0:RPC, EL:2 * EL]

            wph_sb = sload(wph, [H, H], dt=f16, name="c_wph")

            # --- persistent accumulators ------------------------------------
            V_all = singles.tile([H, B * EL], f16, tag="V_all")
            X1T = singles.tile([H, B * EL], bf16, tag="X1T")

            def elu(ps, bias_sb, out_sb, comb=None, relu_eng=None):
                """out_sb(bf16) = elu(ps + bias) = min(exp(t)-1, relu(t)).
                Exact: for t<=0 exp(t)-1 = elu <= 0 = relu; for t>0
                exp(t)-1 >= t = relu. Overflow-safe (inf loses the min).
                exp and relu are independent -> run on parallel engines."""
                cols = ps.shape[1]
                ex = work.tile([ps.shape[0], cols], f32, tag="elu_ex")
                nc.scalar.activation(ex[:], ps[:], Act.Exp, bias=bias_sb[:])
                rl = work.tile([ps.shape[0], cols], f32, tag="elu_rl")
                if relu_eng is None:
                    nc.scalar.activation(rl[:], ps[:], Act.Relu,
                                         bias=bias_sb[:])
                else:
                    relu_eng.tensor_scalar(
                        out=rl[:], in0=ps[:], scalar1=bias_sb[:],
                        scalar2=0.0, op0=Alu.add, op1=Alu.max)
                (comb or nc.vector).scalar_tensor_tensor(
                    out=out_sb, in0=ex[:], scalar=-1.0, in1=rl[:],
                    op0=Alu.add, op1=Alu.min)

            # ================= per-batch edge pipeline =======================
            def a_front_dve(b, rr, Fs, Fr):
                """G and fused pool+relu (the DVE half of the block front)."""
                G = work.tile([H, N * PT], bf16, tag="G", name="G")
                fr_b = Fr[:, rr * PT:(rr + 1) * PT] \
                    .unsqueeze(1).broadcast_to([H, N, PT])
                nc.vector.tensor_tensor(
                    out=G[:].rearrange("p (n t) -> p n t", t=PT),
                    in0=Fs[:].rearrange("p (n t) -> p n t", t=PT),
                    in1=fr_b, op=Alu.add)
                # fused maxpool(k=2) + relu on DVE: max(G_even, G_odd, 0).
                # The host de-interleaves conv1's time columns (even block
                # then odd block per node), so both pool inputs and the
                # output are packed contiguous runs -> DVE 2x mode.
                Y1 = work.tile([H, N * PL], bf16, tag="Y1", name="Y1")
                G3 = G[:].rearrange("p (n x) -> p n x", x=PT)
                nc.vector.scalar_tensor_tensor(
                    out=Y1[:].rearrange("p (e t) -> p e t", t=PL),
                    in0=G3[:, :, 0:PL],
                    scalar=0.0, in1=G3[:, :, PL:PT],
                    op0=Alu.max, op1=Alu.max)
                return Y1

            def a_front_pe(Y1):
                """conv2 matmuls (queued on PE after the small phase-b
                matmuls so those aren't stuck behind the conv2 burst)."""
                Y1r = Y1[:].rearrange("p (e t) -> p e t", t=PL)
                # conv2 into two 2-bank PSUM tiles (strips padded to 512
                # cols) so the relu drain needs 2 instructions, not 4.
                c2ps = []
                for h in range(2):
                    ps = psumc.tile([H, 1024], f32, tag="c2",
                                    name=f"c2ps{h}")
                    for sl in range(2):
                        st = 2 * h + sl
                        for k in range(5):
                            nc.tensor.matmul(
                                ps[:, sl * 512:sl * 512 + C2STRIP],
                                lhsT=w2_sb[k][:],
                                rhs=Y1r[:, st * C2EDGES:(st + 1) * C2EDGES,
                                        k:k + CT],
                                start=(k == 0), stop=(k == 4))
                    c2ps.append(ps)
                return (c2ps,)

            def a_tail(b, rr, c2ps):
                """relu2 (PSUM drain), logits, direct PSUM->A_t DMAs."""
                Y = work.tile([H, N * CT], bf16, tag="Y", name="Y", bufs=5)
                for h in range(2):
                    nc.scalar.activation(
                        Y[:, h * 2 * C2STRIP:(h + 1) * 2 * C2STRIP]
                        .rearrange("p (s x) -> p s x", x=C2STRIP),
                        c2ps[h][:].rearrange("p (s x) -> p s x",
                                             x=512)[:, :, 0:C2STRIP],
                        Act.Relu, bias=b2p_sb[:])
                A_t = work.tile([N, CT], f32, tag="A_t", name="A_t", bufs=3)
                Lsb = work.tile([1, N * CT], f32, tag="Lsb", name="Lsb")
                # Two logit strips per 2-bank PSUM tile; each pair drained
                # by one copy (alternating Scalar/Pool to balance queues).
                for h2 in range(2):
                    lps = psuml.tile([1, 1024], f32, tag="lp", name="lps")
                    for sl in range(2):
                        st = 2 * h2 + sl
                        nc.tensor.matmul(
                            lps[:, sl * 512:sl * 512 + C2STRIP],
                            lhsT=wa_sb[:],
                            rhs=Y[:, st * C2STRIP:(st + 1) * C2STRIP],
                            start=True, stop=True)
                    nc.scalar.copy(
                        Lsb[:, h2 * 2 * C2STRIP:(h2 + 1) * 2 * C2STRIP]
                        .rearrange("p (s x) -> p s x", x=C2STRIP),
                        lps[:].rearrange("p (s x) -> p s x",
                                         x=512)[:, :, 0:C2STRIP])
                nc.sync.dma_start(out=A_t[:], in_=Lsb[:])
                return Y, A_t

            def sm_stage(b, rr, Y, A_t):
                """Softmax + partition-broadcast. No max-subtraction: the
                attention logits here are O(1), far from fp32 exp range."""
                Ex = work.tile([N, CT], f32, tag="Ex", name="Ex")
                S = work.tile([N, 1], f32, tag="S", name="S")
                nc.scalar.activation(Ex[:], A_t[:], Act.Exp,
                                     accum_out=S[:])
                rz = work.tile([N, 1], f32, tag="rz", name="rz")
                nc.vector.reciprocal(rz[:], S[:])
                A_bf = work.tile([N, CT], bf16, tag="A_bf", name="A_bf")
                nc.vector.tensor_scalar(
                    out=A_bf[:], in0=Ex[:], scalar1=rz[:],
                    scalar2=1.0 / CT, op0=Alu.mult, op1=Alu.mult)
                # Bounce through DRAM to broadcast across partitions. The
                # write and the broadcast-read are issued on different
                # queues so independent per-block streams don't block each
                # other head-of-line.
                A_dram = dpool.tile([1, N * CT], bf16, tag="A_dram",
                                    name="A_dram", bufs=4)
                nc.gpsimd.dma_start(out=A_dram[:], in_=A_bf[:])
                A_bc = work.tile([H, N * CT], bf16, tag="A_bc", name="A_bc",
                                 bufs=5)
                nc.sync.dma_start(
                    out=A_bc[:],
                    in_=A_dram[0:1, :].broadcast_to([H, N * CT]))
                return b, rr, Y, A_bc

            def mt_mult(b, rr, Y, A_bc):
                """Weighted temporal multiply. Runs on the Pool engine for
                some blocks to relieve the DVE; the segmented reduce is
                issued one iteration later (mt_reduce) so the DVE FIFO
                never stalls waiting for the slower Pool multiply."""
                eng = nc.gpsimd if rr in (1, 3) else nc.vector
                Mt = work.tile([H, N * CT], f16, tag="Mt", name="Mt",
                               bufs=3)
                eng.tensor_tensor(out=Mt[:], in0=Y[:], in1=A_bc[:],
                                  op=Alu.mult)
                return b, rr, Mt

            def mt_reduce(b, rr, Mt):
                col0 = (b * RPC + rr) * N
                with nc.allow_low_precision(
                        reason="fp16 attention-pool accum, |terms|<=44"):
                    nc.vector.tensor_reduce(
                        out=V_all[:, col0:col0 + N],
                        in_=Mt[:].rearrange("p (e t) -> p e t", t=CT),
                        axis=mybir.AxisListType.X, op=Alu.add)

            def phase_b_part(b, c0, cw, comb=None):
                """convp (folded post-pooling) + mlp1 for cols [c0, c0+cw)
                of batch b."""
                cs = slice(b * EL + c0, b * EL + c0 + cw)
                zps = psum.tile([H, cw], f32, tag="ps", name="zps")
                nc.tensor.matmul(zps[:], lhsT=wph_sb[:], rhs=V_all[:, cs],
                                 start=True, stop=True)
                xsb = work.tile([H, cw], bf16, tag="xsb", name="xsb")
                nc.scalar.copy(xsb[:], zps[:])
                h1ps = psum.tile([H, cw], f32, tag="ps", name="h1ps")
                nc.tensor.matmul(h1ps[:], lhsT=w11_sb[:], rhs=xsb[:],
                                 start=True, stop=True)
                h1sb = work.tile([H, cw], bf16, tag="h1sb", name="h1sb")
                elu(h1ps, b11_sb, h1sb[:], comb=comb)
                h2ps = psum.tile([H, cw], f32, tag="ps", name="h2ps")
                nc.tensor.matmul(h2ps[:], lhsT=w12_sb[:], rhs=h1sb[:],
                                 start=True, stop=True)
                elu(h2ps, b12_sb, X1T[:, cs], comb=comb)

            h3part = singles.tile([H, 2 * EL], bf16, tag="h3part")

            INC_CHUNKS = [(0, 120), (120, EL - 120)]

            def inc_tr(b, j, incps):
                """Transpose one X1T chunk of batch b and accumulate its
                edge2node partial into incps."""
                c0, cw = INC_CHUNKS[j]
                tps = psum.tile([cw, H], bf16, tag="ps", name="tps")
                nc.tensor.transpose(
                    tps[:], in_=X1T[:, b * EL + c0:b * EL + c0 + cw],
                    identity=ident_sb[:])
                x1e = work3.tile([cw, H], bf16, tag=f"x1e{j}", name="x1e")
                nc.scalar.copy(x1e[:], tps[:])
                rel_chunk = rel_ra_sb if j == 0 else rel_rb_sb
                nc.tensor.matmul(
                    incps[:], lhsT=x1e[:],
                    rhs=rel_chunk[:], start=(j == 0), stop=(j == 1))

            def inc_fin(b, incps):
                """mlp2 on the local RPC nodes + mlp3 weight folding ->
                [RPC, H] gather payload."""
                inc_sb = work3.tile([H, RPC], bf16, tag="inc_sb",
                                    name="inc_sb")
                nc.scalar.copy(inc_sb[:], incps[:])
                m2ps = psum.tile([H, RPC], f32, tag="ps", name="m2ps")
                nc.tensor.matmul(m2ps[:], lhsT=w21_sb[:], rhs=inc_sb[:],
                                 start=True, stop=True)
                m2sb = work3.tile([H, RPC], bf16, tag="m2sb", name="m2sb")
                elu(m2ps, b21_sb, m2sb[:], relu_eng=nc.vector)
                m2ps2 = psum.tile([H, RPC], f32, tag="ps", name="m2ps2")
                nc.tensor.matmul(m2ps2[:], lhsT=w22_sb[:], rhs=m2sb[:],
                                 start=True, stop=True)
                x2t = work3.tile([H, RPC], bf16, tag="x2t", name="x2t")
                elu(m2ps2, b22_sb, x2t[:], relu_eng=nc.vector)
                # qaT = x2^T @ w31a (sender term -> gathered),
                # qbT = x2^T @ w31b (receiver term -> stays local);
                # one merged matmul [RPC, 2H]; lhsT = x2 keeps the gather
                # payload and the rc-gather lhsT at partition 0.
                qps = psum.tile([RPC, 2 * H], f32, tag="ps", name="qps")
                nc.tensor.matmul(qps[:], lhsT=x2t[:], rhs=w31ab_sb[:],
                                 start=True, stop=True)
                qt_sb = work3.tile([RPC, 2 * H], bf16, tag="qt_sb",
                                   name="qt_sb")
                nc.scalar.copy(qt_sb[:], qps[:])
                nc.gpsimd.dma_start(out=cc_in[b], in_=qt_sb[:, 0:H])
                return qt_sb

            def cc_b(b):
                nc.gpsimd.collective_compute(
                    "AllGather", mybir.AluOpType.bypass,
                    replica_groups=[list(range(N_CORES))],
                    ins=[cc_in[b]], outs=[cc_out[b]])

            def h3_pre(b, qt_sb):
                """Pre-collective part of mlp3: receiver + skip terms."""
                h3p = psum.tile([H, EL], f32, tag="ps", name="h3p")
                nc.tensor.matmul(h3p[:], lhsT=qt_sb[:, H:2 * H],
                                 rhs=rel_rT_sb[:], start=True, stop=False)
                nc.tensor.matmul(h3p[:], lhsT=w31c_sb[:],
                                 rhs=X1T[:, b * EL:(b + 1) * EL],
                                 start=False, stop=True)
                nc.scalar.copy(h3part[:, (b % 2) * EL:(b % 2 + 1) * EL],
                               h3p[:])

            def f_dma(b):
                """Prefetch the im2col inputs for batch b. All four batches
                are issued upfront on the gpsimd queue (cheap issue, no
                data-dependent traffic ahead of them)."""
                p1_sb = work.tile([CK, N * PT], bf16, tag="p1_sb",
                                  name="p1_sb", bufs=4)
                nc.gpsimd.dma_start(out=p1_sb[:], in_=p1[b])
                p1r_sb = work.tile([CK, RPC * PT], bf16, tag="p1r_sb",
                                   name="p1r_sb", bufs=4)
                nc.gpsimd.dma_start(out=p1r_sb[:], in_=p1r[b])
                return p1_sb, p1r_sb

            def f_stage(b, p1_sb, p1r_sb):
                """conv1 of the node sequences. Uses single-strip tiles
                from the small "ps" pool so it never contends with conv2's
                psumc buffers (which caused batch-boundary stalls)."""
                Fs = work.tile([H, N * PT], bf16, tag="Fs", name="Fs")
                for s8 in range(N * PT // FSTRIP):
                    fps = psum.tile([H, FSTRIP], f32, tag="ps", name="fps")
                    nc.tensor.matmul(
                        fps[:], lhsT=w1s_sb[:],
                        rhs=p1_sb[:, s8 * FSTRIP:(s8 + 1) * FSTRIP],
                        start=True, stop=True)
                    nc.scalar.copy(
                        Fs[:, s8 * FSTRIP:(s8 + 1) * FSTRIP], fps[:])
                Fr = work.tile([H, RPC * PT], bf16, tag="Fr", name="Fr")
                frps = psum.tile([H, RPC * PT], f32, tag="ps", name="frps")
                nc.tensor.matmul(frps[:], lhsT=w1r_sb[:], rhs=p1r_sb[:],
                                 start=True, stop=True)
                nc.scalar.activation(Fr[:], frps[:], Act.Identity,
                                     bias=b1_sb[:])
                return Fs, Fr

            def tail_stage(b):
                """Post-collective: sender gather + mlp3 + fco + output."""
                qg_sb = work.tile([N, H], bf16, tag="qg", name="qg_sb")
                nc.scalar.dma_start(out=qg_sb[:], in_=cc_out[b])
                h3ps = psum.tile([H, EL], f32, tag="ps", name="h3ps")
                nc.tensor.matmul(h3ps[:], lhsT=qg_sb[:], rhs=rel_sT_sb[:],
                                 start=True, stop=False)
                nc.tensor.matmul(h3ps[:], lhsT=ident_sb[:],
                                 rhs=h3part[:, (b % 2) * EL:(b % 2 + 1) * EL],
                                 start=False, stop=True)
                h3sb = work.tile([H, EL], bf16, tag="h3sb", name="h3sb")
                elu(h3ps, b31_sb, h3sb[:], relu_eng=nc.vector)
                h4ps = psum.tile([H, EL], f32, tag="ps", name="h4ps")
                nc.tensor.matmul(h4ps[:], lhsT=w32_sb[:], rhs=h3sb[:],
                                 start=True, stop=True)
                h4sb = work.tile([H, EL], bf16, tag="h4sb", name="h4sb")
                elu(h4ps, b32_sb, h4sb[:], relu_eng=nc.vector)
                ops = psum.tile([O, EL], f32, tag="ps", name="ops")
                nc.tensor.matmul(ops[:], lhsT=wfco_sb[:], rhs=h4sb[:],
                                 start=True, stop=True)
                osb = work.tile([O, EL], f32, tag="osb", name="osb")
                nc.vector.tensor_scalar_add(osb[:], ops[:], bfco_sb[:])
                nc.scalar.dma_start(out=y[b], in_=osb[:])

            # Software pipeline over edge-blocks:
            #   a_front(i) -> sm(i-1) -> mt(i-2)+convp/mlp1 chunk -> a_tail(i)
            # with per-batch edge2node/mlp2/AllGather and the post-collective
            # tail overlapped under later blocks.
            # Software pipeline (block i processed at iteration i):
            #   a_tail(i) -> sm(i) at iter i+1 -> Mt multiply at iter i+3
            #   -> segmented reduce + convp/mlp1 chunk at iter i+4.
            # The deep lag hides the long cross-engine softmax/broadcast
            # chain; the reduce lags the multiply so the DVE FIFO never
            # waits on the Pool engine.
            from collections import deque
            pend_sm = None
            pend_mults = deque()
            pend_red = None
            FsFr = None
            qts = {}
            p1s = [(p1s0, p1r_s0)] + [f_dma(b) for b in range(1, B)]
            blocks = [(b, rr) for b in range(B) for rr in range(RPC)]

            def do_reduce_and_part(red):
                rb, rrr, Mt = red
                mt_reduce(rb, rrr, Mt)
                phase_b_part(rb, rrr * N, N)

            def batch_tail(bb):
                """Strip-level edge2node -> mlp2 -> AllGather issue."""
                incps = psum.tile([H, RPC], f32, tag="ps", name="incps")
                inc_tr(bb, 0, incps)
                inc_tr(bb, 1, incps)
                qts[bb] = inc_fin(bb, incps)
                cc_b(bb)

            FsFr_next = None
            for i, (b, rr) in enumerate(blocks):
                if i == 0:
                    FsFr = f_stage(0, *p1s[0])
                elif rr == 0:
                    FsFr = FsFr_next
                Y1 = a_front_dve(b, rr, *FsFr)
                if pend_red is not None:
                    do_reduce_and_part(pend_red)
                    pend_red = None
                fr = a_front_pe(Y1)
                if len(pend_mults) >= 2:
                    pend_red = mt_mult(*pend_mults.popleft())
                if pend_sm is not None:
                    pend_mults.append(sm_stage(*pend_sm))
                tiles = a_tail(b, rr, *fr)
                pend_sm = (b, rr) + tiles
                if rr == 3 and b + 1 < B:
                    # conv1 for the next batch, two blocks early: its
                    # matmuls/drains hide under this batch's pipeline.
                    FsFr_next = f_stage(b + 1, *p1s[b + 1])
                if i in (8, 13, 18):
                    batch_tail((i - 8) // 5)
                if i in (9, 14, 19):
                    h3_pre((i - 9) // 5, qts[(i - 9) // 5])
                if i in (12, 17):
                    tail_stage((i - 12) // 5)
            # Endgame: drain the pipeline for batch 3, then its serial
            # tail: edge2node -> mlp2 -> AllGather -> mlp3 -> out.
            m32 = mt_mult(*pend_mults.popleft())                 # (3,2)
            sm_out = sm_stage(*pend_sm)                          # (3,4) sm
            do_reduce_and_part(pend_red)                         # (3,1)
            m33 = mt_mult(*pend_mults.popleft())                 # (3,3)
            tail_stage(2)
            do_reduce_and_part(m32)
            m34 = mt_mult(*sm_out)                               # (3,4)
            do_reduce_and_part(m33)
            do_reduce_and_part(m34)
            incps3 = psum.tile([H, RPC], f32, tag="ps", name="incps")
            inc_tr(3, 0, incps3)
            inc_tr(3, 1, incps3)
            qt = inc_fin(3, incps3)
            cc_b(3)
            h3_pre(3, qt)
            tail_stage(3)

    nc.compile()
    _PROGRAM_CACHE["nc"] = nc
    return nc


def _host_prep(inputs, rel_rec, rel_send, p, edge_of):
    """Build the per-core input maps + (core, local, global) output mapping."""
    x = inputs.astype(np.float32)
    # im2col of the node time-series: P1[b, c*5+k, n*96+t] = x[b, n, t+k, c]
    win = np.lib.stride_tricks.sliding_window_view(x, 5, axis=2)  # [B,N,96,D,5]
    P1 = win.transpose(0, 3, 4, 1, 2).reshape(B, CK, N, PT)
    # De-interleave each node's conv1 time columns (even positions first,
    # then odd) so the kernel's fused maxpool reads packed contiguous runs
    # (DVE 2x mode) instead of stride-2 views.
    perm = np.concatenate([np.arange(0, PT, 2), np.arange(1, PT, 2)])
    P1 = np.ascontiguousarray(P1[:, :, :, perm])

    a1 = (p["bn1_g"] / np.sqrt(p["bn1_v"] + BN_EPS)).astype(np.float32)
    c1 = (p["bn1_b"] - p["bn1_m"] * a1).astype(np.float32)
    a2 = (p["bn2_g"] / np.sqrt(p["bn2_v"] + BN_EPS)).astype(np.float32)
    c2 = (p["bn2_b"] - p["bn2_m"] * a2).astype(np.float32)

    w1 = p["conv1_w"].astype(np.float32)           # [H, 2D, 5]
    # rows ordered c*5+k to match P1
    W1s = w1[:, :D, :].transpose(1, 2, 0).reshape(CK, H)
    W1r = w1[:, D:, :].transpose(1, 2, 0).reshape(CK, H)

    w2f = p["conv2_w"].astype(np.float32) * a1[None, :, None]   # [o,i,k]
    b2p = p["conv2_b"].astype(np.float32) + np.einsum(
        "oik,i->o", p["conv2_w"].astype(np.float32), c1)
    W2k = [w2f[:, :, k].T.copy() for k in range(5)]             # lhsT [i,o]

    wa = (p["conva_w"][0, :, 0].astype(np.float32) * a2)[:, None]  # [H,1]
    WpT = (p["convp_w"][:, :, 0].astype(np.float32) * a2[None, :]).T  # [i,o]
    bpp = p["convp_b"].astype(np.float32) + \
        p["convp_w"][:, :, 0].astype(np.float32) @ c2

    m1w1 = p["mlp1_w1"].astype(np.float32)
    m1w2 = p["mlp1_w2"].astype(np.float32)
    b11 = p["mlp1_b1"].astype(np.float32) + (bpp / CT) @ m1w1
    b12 = p["mlp1_b2"].astype(np.float32)
    m2w1 = p["mlp2_w1"].astype(np.float32)
    m2w2 = p["mlp2_w2"].astype(np.float32)
    W21 = m2w1 / N
    b21 = p["mlp2_b1"].astype(np.float32)
    b22 = p["mlp2_b2"].astype(np.float32)
    m3w1 = p["mlp3_w1"].astype(np.float32)
    m3w2 = p["mlp3_w2"].astype(np.float32)
    b31 = p["mlp3_b1"].astype(np.float32)
    b32 = p["mlp3_b2"].astype(np.float32)
    bfco = p["fco_b"].astype(np.float32)

    wpackT = np.concatenate(
        [WpT, m1w1, m1w2, W21, m2w2, m3w1[0:H],
         m3w1[H:2 * H], m3w1[2 * H:3 * H], m3w2,
         np.eye(H, dtype=np.float32), p["fco_w"]],
        axis=1).astype(BF16)
    bpack = np.zeros((H, 9), np.float32)
    for k, v in enumerate([p["conv1_b"], b2p, b11, b12, b21, b22, b31, b32]):
        bpack[:, k] = v
    bpack[:O, 8] = bfco
    shared = {
        "p1": P1.reshape(B, CK, N * PT).astype(BF16),
        "w1pack": np.concatenate([W1s, W1r], axis=1).astype(BF16),
        "wpackF": np.concatenate(W2k + [wa], axis=1).astype(BF16),
        "wpackT": wpackT,
        "bpack": bpack,
        "wph": WpT.astype(np.float16),
    }

    in_maps = []
    out_map = []  # (core, e_loc, e_glob)
    for c in range(N_CORES):
        recvs = list(range(c * RPC, (c + 1) * RPC))
        relr = np.zeros((EL, RPC), np.float32)
        relsT = np.zeros((N, EL), np.float32)
        relrT = np.zeros((N, EL), np.float32)
        for rr_i, r in enumerate(recvs):
            for s in range(N):
                if s == r:
                    continue
                e_loc = rr_i * N + s
                e_g = edge_of[(r, s)]
                relr[e_loc, rr_i] = 1.0
                relsT[s, e_loc] = 1.0
                relrT[rr_i, e_loc] = 1.0
                out_map.append((c, e_loc, e_g))
        m = dict(shared)
        m["p1r"] = np.ascontiguousarray(
            P1[:, :, recvs, :]).reshape(B, CK, RPC * PT).astype(BF16)
        m["rel_r"] = relr.astype(BF16)
        m["relT"] = np.concatenate([relsT, relrT], axis=1).astype(BF16)
        in_maps.append(m)
    return in_maps, out_map


def kernel(**inputs):
    rel_rec = np.asarray(inputs["rel_rec"])
    rel_send = np.asarray(inputs["rel_send"])
    x = np.asarray(inputs["inputs"])
    p = {k: np.asarray(v) for k, v in inputs.items()
         if k not in ("inputs", "rel_rec", "rel_send")}

    edge_of = _nri_structure(rel_rec, rel_send)
    if edge_of is None or x.shape != (B, N, T, D):
        # Inputs without the NRI one-hot structure: fall back to a plain
        # numpy evaluation (correctness path only).
        return _np_forward(x, rel_rec, rel_send, p).astype(np.float32)

    from concourse.bass_utils import run_bass_kernel_spmd

    nc = _build_program()
    in_maps, out_map = _host_prep(x, rel_rec, rel_send, p, edge_of)
    res = run_bass_kernel_spmd(nc, in_maps, list(range(N_CORES)),
                               trace=TRACE)
    if TRACE:
        global LAST_RESULT
        LAST_RESULT = res

    full = np.empty((B, E, O), np.float32)
    for c, e_loc, e_g in out_map:
        full[:, e_g, :] = res.results[c]["y"][:, :, e_loc]
    return full


# revision 30
# speedup vs baseline: 1.0627x; 1.0290x over previous
"""Trainium2 Bass kernel for the NRI CNNEncoder (gnn_message_passing).

Strategy
--------
8-way shard over the edge dimension E=1560: each core owns 5 receiver nodes
x 40 sender slots (the self-edge is computed as padding and discarded on the
host) = 200 local edges x B=4 batches = 800 edge-sequences per core.

Algebraic restructuring (all exact, eval-mode):
- conv1 is linear, so per-edge conv1(concat(send, recv)) = F_s[send] + F_r[recv]
  where F_s/F_r are convolutions of the 160 node sequences with the two halves
  of conv1_w: a 39x compute reduction on conv1.
- BatchNorm (eval) = per-channel positive-scale affine; it commutes with
  maxpool and folds into the following conv/matmul weights (bn1 -> conv2,
  bn2 -> convp/conva).
- convp (1x1) commutes with the attention-weighted temporal pooling, so it is
  applied after pooling: 44x less convp compute.
- ELU is stored shifted: elu(t)+1 = min(exp(t), 1+relu(t)) — two parallel
  engine ops + one combine; the "+1" is folded into the next layer's bias on
  the host.
- edge2node is local per receiver shard; the node2edge gather needs one
  per-batch AllGather of the mlp2 outputs (1.25KB). A dummy AllGather at
  kernel start absorbs the ~50us first-use collective setup.

Matmuls run in bf16 (fp32 PSUM accumulation); softmax in fp32.
"""

import os
import sys
import numpy as np

sys.path.insert(0, "/opt/trn_rl_repo")

import ml_dtypes

BF16 = ml_dtypes.bfloat16

# Problem constants (hardcoded; must match the reference).
B, N, T, D, H, O = 4, 40, 100, 4, 128, 2
E = N * (N - 1)          # 1560
BN_EPS = 1e-5
N_CORES = 8
RPC = N // N_CORES       # receivers per core = 5
EL = RPC * N             # local edges per core (incl. self padding) = 200
PT = T - 4               # conv1 output length = 96
PL = PT // 2             # pooled length = 48
CT = PL - 4              # conv2 output length = 44
CK = 20                  # conv1 contraction = D * K = 4*5
FSTRIP = 480             # F matmul strip (5 nodes x 96)
C2EDGES = 10             # edges per conv2/logits strip
C2STRIP = C2EDGES * CT   # 440


def _np_forward(inputs, rel_rec, rel_send, p):
    """Pure-numpy fp32 replica of the reference (fallback for inputs whose
    rel matrices do not have the NRI one-hot structure)."""
    x32 = inputs.astype(np.float32)
    rr = rel_rec.astype(np.float32)
    rs = rel_send.astype(np.float32)
    xf = x32.reshape(B, N, T * D)
    recv = np.einsum("en,bnf->bef", rr, xf).reshape(B * rr.shape[0], T, D)
    send = np.einsum("en,bnf->bef", rs, xf).reshape(B * rs.shape[0], T, D)
    x = np.concatenate([send.transpose(0, 2, 1), recv.transpose(0, 2, 1)], axis=1)

    def conv1d(x, w, b):
        k = w.shape[2]
        t_out = x.shape[2] - k + 1
        y = np.zeros((x.shape[0], w.shape[0], t_out), np.float32)
        for kk in range(k):
            y += np.einsum("oc,nct->not", w[:, :, kk], x[:, :, kk:kk + t_out])
        return y + b[None, :, None]

    def bn(x, g, b, m, v):
        return (x - m[None, :, None]) / np.sqrt(v[None, :, None] + BN_EPS) \
            * g[None, :, None] + b[None, :, None]

    def elu(x):
        return np.where(x > 0, x, np.expm1(x))

    def mlp(x, w1, b1, w2, b2):
        h = elu(x @ w1 + b1)
        return elu(h @ w2 + b2)

    x = bn(np.maximum(conv1d(x, p["conv1_w"], p["conv1_b"]), 0.0),
           p["bn1_g"], p["bn1_b"], p["bn1_m"], p["bn1_v"])
    n_, c_, t_ = x.shape
    x = x.reshape(n_, c_, t_ // 2, 2).max(axis=-1)
    x = bn(np.maximum(conv1d(x, p["conv2_w"], p["conv2_b"]), 0.0),
           p["bn2_g"], p["bn2_b"], p["bn2_m"], p["bn2_v"])
    pred = conv1d(x, p["convp_w"], p["convp_b"])
    a = conv1d(x, p["conva_w"], p["conva_b"])
    a = np.exp(a - a.max(axis=2, keepdims=True))
    a = a / a.sum(axis=2, keepdims=True)
    x = (pred * a).mean(axis=2).reshape(B, -1, H)
    x = mlp(x, p["mlp1_w1"], p["mlp1_b1"], p["mlp1_w2"], p["mlp1_b2"])
    x_skip = x
    inc = np.einsum("en,beh->bnh", rr, x) / N
    x = mlp(inc, p["mlp2_w1"], p["mlp2_b1"], p["mlp2_w2"], p["mlp2_b2"])
    sn = np.einsum("en,bnh->beh", rs, x)
    rc = np.einsum("en,bnh->beh", rr, x)
    x = np.concatenate([sn, rc, x_skip], axis=2)
    x = mlp(x, p["mlp3_w1"], p["mlp3_b1"], p["mlp3_w2"], p["mlp3_b2"])
    return x @ p["fco_w"] + p["fco_b"]


def _nri_structure(rel_rec, rel_send):
    """If (rel_rec, rel_send) are the NRI fully-connected one-hot matrices,
    return edge_of[r][s] -> global edge index; else None."""
    if rel_rec.shape != (E, N) or rel_send.shape != (E, N):
        return None
    rec_i = np.argmax(rel_rec, axis=1)
    snd_i = np.argmax(rel_send, axis=1)
    eye = np.eye(N, dtype=rel_rec.dtype)
    if not (np.array_equal(rel_rec, eye[rec_i]) and
            np.array_equal(rel_send, eye[snd_i])):
        return None
    edge_of = {}
    for e in range(E):
        r, s = int(rec_i[e]), int(snd_i[e])
        if r == s or (r, s) in edge_of:
            return None
        edge_of[(r, s)] = e
    if len(edge_of) != E:
        return None
    return edge_of


_PROGRAM_CACHE = {}
TRACE = False          # test harness sets True to collect NTFF exec time
LAST_RESULT = None     # BassKernelResults of the last run (when TRACE)


def _build_program():
    """Build + compile the SPMD Bass program (cached per process)."""
    if "nc" in _PROGRAM_CACHE:
        return _PROGRAM_CACHE["nc"]

    import concourse.bacc as bacc
    import concourse.tile as tile
    from concourse import mybir
    from contextlib import ExitStack

    f32 = mybir.dt.float32
    f16 = mybir.dt.float16
    bf16 = mybir.dt.bfloat16
    Alu = mybir.AluOpType
    Act = mybir.ActivationFunctionType

    nc = bacc.Bacc("TRN2", target_bir_lowering=False, debug=False,
                   num_devices=N_CORES)

    def din(name, shape, dt=bf16):
        return nc.dram_tensor(name, shape, dt, kind="ExternalInput").ap()

    p1 = din("p1", [B, CK, N * PT])
    p1r = din("p1r", [B, CK, RPC * PT])
    rel_r = din("rel_r", [EL, RPC])
    w1pack = din("w1pack", [CK, 2 * H])
    wpackF = din("wpackF", [H, 5 * H + 1])
    wpackT = din("wpackT", [H, 10 * H + O])
    bpack = din("bpack", [H, 9], f32)
    relT = din("relT", [N, 2 * EL])
    wph = din("wph", [H, H], f16)

    y = nc.dram_tensor("y", [B, O, EL], f32, kind="ExternalOutput").ap()
    # AllGather payload: per-core [RPC, H] bf16 (only q_a = w31a^T @ x2 is
    # gathered; the receiver term stays local).
    cc_in = nc.dram_tensor("cc_in", [B, RPC, H], bf16).ap()
    cc_out = nc.dram_tensor("cc_out", [B, N, H], bf16,
                        addr_space="Shared").ap()
    # Dummy collective to absorb the ~50us first-use setup cost.
    wu_in = nc.dram_tensor("wu_in", [1, 4], bf16).ap()
    wu_out = nc.dram_tensor("wu_out", [N_CORES, 4], bf16,
                            addr_space="Shared").ap()

    with tile.TileContext(nc) as tc:
        with ExitStack() as ctx:
            singles = ctx.enter_context(tc.tile_pool(name="singles", bufs=1))
            work = ctx.enter_context(tc.tile_pool(name="work", bufs=2))
            work3 = ctx.enter_context(tc.tile_pool(name="work3", bufs=3))
            psum = ctx.enter_context(
                tc.tile_pool(name="psum", bufs=2, space="PSUM"))
            psumc = ctx.enter_context(
                tc.tile_pool(name="psumc", bufs=2, space="PSUM"))
            psuml = ctx.enter_context(
                tc.tile_pool(name="psuml", bufs=1, space="PSUM"))
            dpool = ctx.enter_context(
                tc.tile_pool(name="dpool", bufs=2, space="DRAM"))

            def sload(ap_dram, shape, dt=bf16, name=None, eng=None):
                t = singles.tile(shape, dt,
                                 name=name or f"c_{ap_dram.tensor.name}")
                (eng or nc.sync).dma_start(out=t[:], in_=ap_dram)
                return t

            # --- collective warm-up (first thing on the CC queue) --------
            wu_sb = singles.tile([1, 4], bf16, name="wu_sb")
            nc.vector.memset(wu_sb[:], 0.0)
            nc.gpsimd.dma_start(out=wu_in, in_=wu_sb[:])
            nc.gpsimd.collective_compute(
                "AllGather", mybir.AluOpType.bypass,
                replica_groups=[list(range(N_CORES))],
                ins=[wu_in], outs=[wu_out])

            # --- weights / constants into SBUF (packed DMAs) ------------
            # w1pack + batch 0's im2col inputs first: the first f_stage
            # needs exactly these, everything else can trickle in after.
            w1pack_sb = sload(w1pack, [CK, 2 * H])
            p1s0 = work.tile([CK, N * PT], bf16, tag="p1_sb",
                             name="p1_sb", bufs=4)
            nc.sync.dma_start(out=p1s0[:], in_=p1[0])
            p1r_s0 = work.tile([CK, RPC * PT], bf16, tag="p1r_sb",
                               name="p1r_sb", bufs=4)
            nc.sync.dma_start(out=p1r_s0[:], in_=p1r[0])
            w1s_sb = w1pack_sb[:, 0:H]
            w1r_sb = w1pack_sb[:, H:2 * H]
            wpackF_sb = sload(wpackF, [H, 5 * H + 1])
            w2_sb = [wpackF_sb[:, k * H:(k + 1) * H] for k in range(5)]
            wa_sb = wpackF_sb[:, 5 * H:5 * H + 1]
            wpackT_sb = sload(wpackT, [H, 10 * H + O], eng=nc.scalar)
            (wp_sb, w11_sb, w12_sb, w21_sb, w22_sb, w31a_sb, w31b_sb,
             w31c_sb, w32_sb, ident_sb) = [
                wpackT_sb[:, k * H:(k + 1) * H] for k in range(10)]
            w31ab_sb = wpackT_sb[:, 5 * H:7 * H]
            wfco_sb = wpackT_sb[:, 10 * H:10 * H + O]
            bpack_sb = sload(bpack, [H, 9], f32)
            (b1_sb, b2p_sb, b11_sb, b12_sb, b21_sb, b22_sb, b31_sb,
             b32_sb) = [bpack_sb[:, k:k + 1] for k in range(8)]
            bfco_sb = bpack_sb[0:O, 8:9]
            rel_ra_sb = sload(rel_r[0:120, :], [120, RPC], name="rel_ra",
                              eng=nc.gpsimd)
            rel_rb_sb = sload(rel_r[120:EL, :], [EL - 120, RPC],
                              name="rel_rb", eng=nc.gpsimd)
            relT_sb = sload(relT, [N, 2 * EL], eng=nc.gpsimd)
            rel_sT_sb = relT_sb[:, 0:EL]
            rel_rT_sb = relT_sb[0:RPC, EL:2 * EL]

            wph_sb = sload(wph, [H, H], dt=f16, name="c_wph")

            # --- persistent accumulators ------------------------------------
            V_all = singles.tile([H, B * EL], f16, tag="V_all")
            X1T = singles.tile([H, B * EL], bf16, tag="X1T")

            def elu(ps, bias_sb, out_sb, comb=None, relu_eng=None):
                """out_sb(bf16) = elu(ps + bias) = min(exp(t)-1, relu(t)).
                Exact: for t<=0 exp(t)-1 = elu <= 0 = relu; for t>0
                exp(t)-1 >= t = relu. Overflow-safe (inf loses the min).
                exp and relu are independent -> run on parallel engines."""
                cols = ps.shape[1]
                ex = work.tile([ps.shape[0], cols], f32, tag="elu_ex")
                nc.scalar.activation(ex[:], ps[:], Act.Exp, bias=bias_sb[:])
                rl = work.tile([ps.shape[0], cols], f32, tag="elu_rl")
                if relu_eng is None:
                    nc.scalar.activation(rl[:], ps[:], Act.Relu,
                                         bias=bias_sb[:])
                else:
                    relu_eng.tensor_scalar(
                        out=rl[:], in0=ps[:], scalar1=bias_sb[:],
                        scalar2=0.0, op0=Alu.add, op1=Alu.max)
                (comb or nc.vector).scalar_tensor_tensor(
                    out=out_sb, in0=ex[:], scalar=-1.0, in1=rl[:],
                    op0=Alu.add, op1=Alu.min)

            # ================= per-batch edge pipeline =======================
            def a_front_dve(b, rr, Fs, Fr):
                """G and fused pool+relu (the DVE half of the block front)."""
                G = work.tile([H, N * PT], bf16, tag="G", name="G")
                fr_b = Fr[:, rr * PT:(rr + 1) * PT] \
                    .unsqueeze(1).broadcast_to([H, N, PT])
                nc.vector.tensor_tensor(
                    out=G[:].rearrange("p (n t) -> p n t", t=PT),
                    in0=Fs[:].rearrange("p (n t) -> p n t", t=PT),
                    in1=fr_b, op=Alu.add)
                # fused maxpool(k=2) + relu on DVE: max(G_even, G_odd, 0).
                # The host de-interleaves conv1's time columns (even block
                # then odd block per node), so both pool inputs and the
                # output are packed contiguous runs -> DVE 2x mode.
                Y1 = work.tile([H, N * PL], bf16, tag="Y1", name="Y1")
                G3 = G[:].rearrange("p (n x) -> p n x", x=PT)
                nc.vector.scalar_tensor_tensor(
                    out=Y1[:].rearrange("p (e t) -> p e t", t=PL),
                    in0=G3[:, :, 0:PL],
                    scalar=0.0, in1=G3[:, :, PL:PT],
                    op0=Alu.max, op1=Alu.max)
                return Y1

            def a_front_pe(Y1):
                """conv2 matmuls (queued on PE after the small phase-b
                matmuls so those aren't stuck behind the conv2 burst)."""
                Y1r = Y1[:].rearrange("p (e t) -> p e t", t=PL)
                # conv2 into two 2-bank PSUM tiles (strips padded to 512
                # cols) so the relu drain needs 2 instructions, not 4.
                c2ps = []
                for h in range(2):
                    ps = psumc.tile([H, 1024], f32, tag="c2",
                                    name=f"c2ps{h}")
                    for sl in range(2):
                        st = 2 * h + sl
                        for k in range(5):
                            nc.tensor.matmul(
                                ps[:, sl * 512:sl * 512 + C2STRIP],
                                lhsT=w2_sb[k][:],
                                rhs=Y1r[:, st * C2EDGES:(st + 1) * C2EDGES,
                                        k:k + CT],
                                start=(k == 0), stop=(k == 4))
                    c2ps.append(ps)
                return (c2ps,)

            def a_tail(b, rr, c2ps):
                """relu2 (PSUM drain), logits, direct PSUM->A_t DMAs."""
                Y = work.tile([H, N * CT], bf16, tag="Y", name="Y", bufs=6)
                for h in range(2):
                    nc.scalar.activation(
                        Y[:, h * 2 * C2STRIP:(h + 1) * 2 * C2STRIP]
                        .rearrange("p (s x) -> p s x", x=C2STRIP),
                        c2ps[h][:].rearrange("p (s x) -> p s x",
                                             x=512)[:, :, 0:C2STRIP],
                        Act.Relu, bias=b2p_sb[:])
                A_t = work.tile([N, CT], f32, tag="A_t", name="A_t", bufs=4)
                Lsb = work.tile([1, N * CT], f32, tag="Lsb", name="Lsb",
                                bufs=3)
                # Two logit strips per 2-bank PSUM tile; each pair drained
                # by one copy (alternating Scalar/Pool to balance queues).
                for h2 in range(2):
                    lps = psuml.tile([1, 1024], f32, tag="lp", name="lps")
                    for sl in range(2):
                        st = 2 * h2 + sl
                        nc.tensor.matmul(
                            lps[:, sl * 512:sl * 512 + C2STRIP],
                            lhsT=wa_sb[:],
                            rhs=Y[:, st * C2STRIP:(st + 1) * C2STRIP],
                            start=True, stop=True)
                    nc.scalar.copy(
                        Lsb[:, h2 * 2 * C2STRIP:(h2 + 1) * 2 * C2STRIP]
                        .rearrange("p (s x) -> p s x", x=C2STRIP),
                        lps[:].rearrange("p (s x) -> p s x",
                                         x=512)[:, :, 0:C2STRIP])
                nc.sync.dma_start(out=A_t[:], in_=Lsb[:])
                return Y, A_t

            def sm_stage(b, rr, Y, A_t):
                """Softmax + partition-broadcast. No max-subtraction: the
                attention logits here are O(1), far from fp32 exp range."""
                Ex = work.tile([N, CT], f32, tag="Ex", name="Ex")
                S = work.tile([N, 1], f32, tag="S", name="S")
                nc.scalar.activation(Ex[:], A_t[:], Act.Exp,
                                     accum_out=S[:])
                rz = work.tile([N, 1], f32, tag="rz", name="rz")
                nc.vector.reciprocal(rz[:], S[:])
                # Normalize on Scalar (copy with per-partition scale); the
                # 1/CT of the temporal mean is folded into wph on the host.
                A_bf = work.tile([N, CT], bf16, tag="A_bf", name="A_bf")
                nc.scalar.activation(A_bf[:], Ex[:], Act.Copy,
                                     scale=rz[:])
                # Bounce through DRAM to broadcast across partitions. The
                # write and the broadcast-read are issued on different
                # queues so independent per-block streams don't block each
                # other head-of-line.
                A_dram = dpool.tile([1, N * CT], bf16, tag="A_dram",
                                    name="A_dram", bufs=5)
                nc.gpsimd.dma_start(out=A_dram[:], in_=A_bf[:])
                A_bc = work.tile([H, N * CT], bf16, tag="A_bc", name="A_bc",
                                 bufs=6)
                nc.sync.dma_start(
                    out=A_bc[:],
                    in_=A_dram[0:1, :].broadcast_to([H, N * CT]))
                return b, rr, Y, A_bc

            def mt_mult(b, rr, Y, A_bc):
                """Weighted temporal multiply. Runs on the Pool engine for
                some blocks to relieve the DVE; the segmented reduce is
                issued one iteration later (mt_reduce) so the DVE FIFO
                never stalls waiting for the slower Pool multiply."""
                eng = nc.gpsimd if rr in (1, 3) else nc.vector
                Mt = work.tile([H, N * CT], f16, tag="Mt", name="Mt",
                               bufs=3)
                eng.tensor_tensor(out=Mt[:], in0=Y[:], in1=A_bc[:],
                                  op=Alu.mult)
                return b, rr, Mt

            def mt_reduce(b, rr, Mt):
                col0 = (b * RPC + rr) * N
                with nc.allow_low_precision(
                        reason="fp16 attention-pool accum, |terms|<=44"):
                    nc.vector.tensor_reduce(
                        out=V_all[:, col0:col0 + N],
                        in_=Mt[:].rearrange("p (e t) -> p e t", t=CT),
                        axis=mybir.AxisListType.X, op=Alu.add)

            def phase_b_part(b, c0, cw, comb=None):
                """convp (folded post-pooling) + mlp1 for cols [c0, c0+cw)
                of batch b."""
                cs = slice(b * EL + c0, b * EL + c0 + cw)
                zps = psum.tile([H, cw], f32, tag="ps", name="zps")
                nc.tensor.matmul(zps[:], lhsT=wph_sb[:], rhs=V_all[:, cs],
                                 start=True, stop=True)
                xsb = work.tile([H, cw], bf16, tag="xsb", name="xsb")
                nc.scalar.copy(xsb[:], zps[:])
                h1ps = psum.tile([H, cw], f32, tag="ps", name="h1ps")
                nc.tensor.matmul(h1ps[:], lhsT=w11_sb[:], rhs=xsb[:],
                                 start=True, stop=True)
                h1sb = work.tile([H, cw], bf16, tag="h1sb", name="h1sb")
                elu(h1ps, b11_sb, h1sb[:], comb=comb)
                h2ps = psum.tile([H, cw], f32, tag="ps", name="h2ps")
                nc.tensor.matmul(h2ps[:], lhsT=w12_sb[:], rhs=h1sb[:],
                                 start=True, stop=True)
                elu(h2ps, b12_sb, X1T[:, cs], comb=comb)

            h3part = singles.tile([H, 2 * EL], bf16, tag="h3part")

            INC_CHUNKS = [(0, 120), (120, EL - 120)]

            def inc_tr(b, j, incps):
                """Transpose one X1T chunk of batch b and accumulate its
                edge2node partial into incps."""
                c0, cw = INC_CHUNKS[j]
                tps = psum.tile([cw, H], bf16, tag="ps", name="tps")
                nc.tensor.transpose(
                    tps[:], in_=X1T[:, b * EL + c0:b * EL + c0 + cw],
                    identity=ident_sb[:])
                x1e = work3.tile([cw, H], bf16, tag=f"x1e{j}", name="x1e")
                nc.scalar.copy(x1e[:], tps[:])
                rel_chunk = rel_ra_sb if j == 0 else rel_rb_sb
                nc.tensor.matmul(
                    incps[:], lhsT=x1e[:],
                    rhs=rel_chunk[:], start=(j == 0), stop=(j == 1))

            def inc_fin(b, incps):
                """mlp2 on the local RPC nodes + mlp3 weight folding ->
                [RPC, H] gather payload."""
                inc_sb = work3.tile([H, RPC], bf16, tag="inc_sb",
                                    name="inc_sb")
                nc.scalar.copy(inc_sb[:], incps[:])
                m2ps = psum.tile([H, RPC], f32, tag="ps", name="m2ps")
                nc.tensor.matmul(m2ps[:], lhsT=w21_sb[:], rhs=inc_sb[:],
                                 start=True, stop=True)
                m2sb = work3.tile([H, RPC], bf16, tag="m2sb", name="m2sb")
                elu(m2ps, b21_sb, m2sb[:], relu_eng=nc.vector)
                m2ps2 = psum.tile([H, RPC], f32, tag="ps", name="m2ps2")
                nc.tensor.matmul(m2ps2[:], lhsT=w22_sb[:], rhs=m2sb[:],
                                 start=True, stop=True)
                x2t = work3.tile([H, RPC], bf16, tag="x2t", name="x2t")
                elu(m2ps2, b22_sb, x2t[:], relu_eng=nc.vector)
                # qaT = x2^T @ w31a (sender term -> gathered),
                # qbT = x2^T @ w31b (receiver term -> stays local);
                # one merged matmul [RPC, 2H]; lhsT = x2 keeps the gather
                # payload and the rc-gather lhsT at partition 0.
                qps = psum.tile([RPC, 2 * H], f32, tag="ps", name="qps")
                nc.tensor.matmul(qps[:], lhsT=x2t[:], rhs=w31ab_sb[:],
                                 start=True, stop=True)
                qt_sb = work3.tile([RPC, 2 * H], bf16, tag="qt_sb",
                                   name="qt_sb")
                nc.scalar.copy(qt_sb[:], qps[:])
                nc.gpsimd.dma_start(out=cc_in[b], in_=qt_sb[:, 0:H])
                return qt_sb

            def cc_b(b):
                nc.gpsimd.collective_compute(
                    "AllGather", mybir.AluOpType.bypass,
                    replica_groups=[list(range(N_CORES))],
                    ins=[cc_in[b]], outs=[cc_out[b]])

            def h3_pre(b, qt_sb):
                """Pre-collective part of mlp3: receiver + skip terms."""
                h3p = psum.tile([H, EL], f32, tag="ps", name="h3p")
                nc.tensor.matmul(h3p[:], lhsT=qt_sb[:, H:2 * H],
                                 rhs=rel_rT_sb[:], start=True, stop=False)
                nc.tensor.matmul(h3p[:], lhsT=w31c_sb[:],
                                 rhs=X1T[:, b * EL:(b + 1) * EL],
                                 start=False, stop=True)
                nc.scalar.copy(h3part[:, (b % 2) * EL:(b % 2 + 1) * EL],
                               h3p[:])

            def f_dma(b):
                """Prefetch the im2col inputs for batch b. All four batches
                are issued upfront on the gpsimd queue (cheap issue, no
                data-dependent traffic ahead of them)."""
                p1_sb = work.tile([CK, N * PT], bf16, tag="p1_sb",
                                  name="p1_sb", bufs=4)
                nc.gpsimd.dma_start(out=p1_sb[:], in_=p1[b])
                p1r_sb = work.tile([CK, RPC * PT], bf16, tag="p1r_sb",
                                   name="p1r_sb", bufs=4)
                nc.gpsimd.dma_start(out=p1r_sb[:], in_=p1r[b])
                return p1_sb, p1r_sb

            def f_stage(b, p1_sb, p1r_sb):
                """conv1 of the node sequences. Uses single-strip tiles
                from the small "ps" pool so it never contends with conv2's
                psumc buffers (which caused batch-boundary stalls)."""
                Fs = work.tile([H, N * PT], bf16, tag="Fs", name="Fs")
                for s8 in range(N * PT // FSTRIP):
                    fps = psum.tile([H, FSTRIP], f32, tag="ps", name="fps")
                    nc.tensor.matmul(
                        fps[:], lhsT=w1s_sb[:],
                        rhs=p1_sb[:, s8 * FSTRIP:(s8 + 1) * FSTRIP],
                        start=True, stop=True)
                    nc.scalar.copy(
                        Fs[:, s8 * FSTRIP:(s8 + 1) * FSTRIP], fps[:])
                Fr = work.tile([H, RPC * PT], bf16, tag="Fr", name="Fr")
                frps = psum.tile([H, RPC * PT], f32, tag="ps", name="frps")
                nc.tensor.matmul(frps[:], lhsT=w1r_sb[:], rhs=p1r_sb[:],
                                 start=True, stop=True)
                nc.scalar.activation(Fr[:], frps[:], Act.Identity,
                                     bias=b1_sb[:])
                return Fs, Fr

            def tail_stage(b):
                """Post-collective: sender gather + mlp3 + fco + output."""
                qg_sb = work.tile([N, H], bf16, tag="qg", name="qg_sb")
                nc.scalar.dma_start(out=qg_sb[:], in_=cc_out[b])
                h3ps = psum.tile([H, EL], f32, tag="ps", name="h3ps")
                nc.tensor.matmul(h3ps[:], lhsT=qg_sb[:], rhs=rel_sT_sb[:],
                                 start=True, stop=False)
                nc.tensor.matmul(h3ps[:], lhsT=ident_sb[:],
                                 rhs=h3part[:, (b % 2) * EL:(b % 2 + 1) * EL],
                                 start=False, stop=True)
                h3sb = work.tile([H, EL], bf16, tag="h3sb", name="h3sb")
                elu(h3ps, b31_sb, h3sb[:], relu_eng=nc.vector)
                h4ps = psum.tile([H, EL], f32, tag="ps", name="h4ps")
                nc.tensor.matmul(h4ps[:], lhsT=w32_sb[:], rhs=h3sb[:],
                                 start=True, stop=True)
                h4sb = work.tile([H, EL], bf16, tag="h4sb", name="h4sb")
                elu(h4ps, b32_sb, h4sb[:], relu_eng=nc.vector)
                ops = psum.tile([O, EL], f32, tag="ps", name="ops")
                nc.tensor.matmul(ops[:], lhsT=wfco_sb[:], rhs=h4sb[:],
                                 start=True, stop=True)
                osb = work.tile([O, EL], f32, tag="osb", name="osb")
                nc.vector.tensor_scalar_add(osb[:], ops[:], bfco_sb[:])
                nc.scalar.dma_start(out=y[b], in_=osb[:])

            # Software pipeline over edge-blocks:
            #   a_front(i) -> sm(i-1) -> mt(i-2)+convp/mlp1 chunk -> a_tail(i)
            # with per-batch edge2node/mlp2/AllGather and the post-collective
            # tail overlapped under later blocks.
            # Software pipeline (block i processed at iteration i):
            #   a_tail(i) -> sm(i) at iter i+2 -> Mt multiply at iter i+4
            #   -> segmented reduce + convp/mlp1 chunk at iter i+5.
            # Every cross-engine hop gets at least a full iteration of
            # slack so the strictly-FIFO engine queues never stall on a
            # not-yet-ready dependency; the reduce lags the multiply so
            # the DVE never waits on the Pool engine.
            from collections import deque
            pend_sms = deque()
            pend_mults = deque()
            pend_red = None
            FsFr = None
            qts = {}
            p1s = [(p1s0, p1r_s0)] + [f_dma(b) for b in range(1, B)]
            blocks = [(b, rr) for b in range(B) for rr in range(RPC)]

            def do_reduce_and_part(red):
                rb, rrr, Mt = red
                mt_reduce(rb, rrr, Mt)
                phase_b_part(rb, rrr * N, N)

            def batch_tail(bb):
                """Strip-level edge2node -> mlp2 -> AllGather issue."""
                incps = psum.tile([H, RPC], f32, tag="ps", name="incps")
                inc_tr(bb, 0, incps)
                inc_tr(bb, 1, incps)
                qts[bb] = inc_fin(bb, incps)
                cc_b(bb)

            FsFr_next = None
            for i, (b, rr) in enumerate(blocks):
                if i == 0:
                    FsFr = f_stage(0, *p1s[0])
                elif rr == 0:
                    FsFr = FsFr_next
                Y1 = a_front_dve(b, rr, *FsFr)
                if pend_red is not None:
                    do_reduce_and_part(pend_red)
                    pend_red = None
                fr = a_front_pe(Y1)
                if len(pend_mults) >= 2:
                    pend_red = mt_mult(*pend_mults.popleft())
                if len(pend_sms) >= 2:
                    pend_mults.append(sm_stage(*pend_sms.popleft()))
                tiles = a_tail(b, rr, *fr)
                pend_sms.append((b, rr) + tiles)
                if rr == 3 and b + 1 < B:
                    # conv1 for the next batch, two blocks early: its
                    # matmuls/drains hide under this batch's pipeline.
                    FsFr_next = f_stage(b + 1, *p1s[b + 1])
                if i in (9, 14, 19):
                    batch_tail((i - 9) // 5)
                if i in (10, 15):
                    h3_pre((i - 10) // 5, qts[(i - 10) // 5])
                if i in (12, 17):
                    tail_stage((i - 12) // 5)
            # Endgame: drain the 5-stage pipeline for batch 3, then its
            # serial tail: edge2node -> mlp2 -> AllGather -> mlp3 -> out.
            do_reduce_and_part(pend_red)                         # (3,0)
            m = mt_mult(*pend_mults.popleft())                   # (3,1)
            pend_mults.append(sm_stage(*pend_sms.popleft()))     # sm(3,3)
            do_reduce_and_part(m)
            m = mt_mult(*pend_mults.popleft())                   # (3,2)
            pend_mults.append(sm_stage(*pend_sms.popleft()))     # sm(3,4)
            h3_pre(2, qts[2])
            do_reduce_and_part(m)
            m = mt_mult(*pend_mults.popleft())                   # (3,3)
            do_reduce_and_part(m)
            m = mt_mult(*pend_mults.popleft())                   # (3,4)
            tail_stage(2)
            do_reduce_and_part(m)
            incps3 = psum.tile([H, RPC], f32, tag="ps", name="incps")
            inc_tr(3, 0, incps3)
            inc_tr(3, 1, incps3)
            qt = inc_fin(3, incps3)
            cc_b(3)
            h3_pre(3, qt)
            tail_stage(3)

    nc.compile()
    _PROGRAM_CACHE["nc"] = nc
    return nc


def _host_prep(inputs, rel_rec, rel_send, p, edge_of):
    """Build the per-core input maps + (core, local, global) output mapping."""
    x = inputs.astype(np.float32)
    # im2col of the node time-series: P1[b, c*5+k, n*96+t] = x[b, n, t+k, c]
    win = np.lib.stride_tricks.sliding_window_view(x, 5, axis=2)  # [B,N,96,D,5]
    P1 = win.transpose(0, 3, 4, 1, 2).reshape(B, CK, N, PT)
    # De-interleave each node's conv1 time columns (even positions first,
    # then odd) so the kernel's fused maxpool reads packed contiguous runs
    # (DVE 2x mode) instead of stride-2 views.
    perm = np.concatenate([np.arange(0, PT, 2), np.arange(1, PT, 2)])
    P1 = np.ascontiguousarray(P1[:, :, :, perm])

    a1 = (p["bn1_g"] / np.sqrt(p["bn1_v"] + BN_EPS)).astype(np.float32)
    c1 = (p["bn1_b"] - p["bn1_m"] * a1).astype(np.float32)
    a2 = (p["bn2_g"] / np.sqrt(p["bn2_v"] + BN_EPS)).astype(np.float32)
    c2 = (p["bn2_b"] - p["bn2_m"] * a2).astype(np.float32)

    w1 = p["conv1_w"].astype(np.float32)           # [H, 2D, 5]
    # rows ordered c*5+k to match P1
    W1s = w1[:, :D, :].transpose(1, 2, 0).reshape(CK, H)
    W1r = w1[:, D:, :].transpose(1, 2, 0).reshape(CK, H)

    w2f = p["conv2_w"].astype(np.float32) * a1[None, :, None]   # [o,i,k]
    b2p = p["conv2_b"].astype(np.float32) + np.einsum(
        "oik,i->o", p["conv2_w"].astype(np.float32), c1)
    W2k = [w2f[:, :, k].T.copy() for k in range(5)]             # lhsT [i,o]

    wa = (p["conva_w"][0, :, 0].astype(np.float32) * a2)[:, None]  # [H,1]
    WpT = (p["convp_w"][:, :, 0].astype(np.float32) * a2[None, :]).T  # [i,o]
    bpp = p["convp_b"].astype(np.float32) + \
        p["convp_w"][:, :, 0].astype(np.float32) @ c2

    m1w1 = p["mlp1_w1"].astype(np.float32)
    m1w2 = p["mlp1_w2"].astype(np.float32)
    b11 = p["mlp1_b1"].astype(np.float32) + (bpp / CT) @ m1w1
    b12 = p["mlp1_b2"].astype(np.float32)
    m2w1 = p["mlp2_w1"].astype(np.float32)
    m2w2 = p["mlp2_w2"].astype(np.float32)
    W21 = m2w1 / N
    b21 = p["mlp2_b1"].astype(np.float32)
    b22 = p["mlp2_b2"].astype(np.float32)
    m3w1 = p["mlp3_w1"].astype(np.float32)
    m3w2 = p["mlp3_w2"].astype(np.float32)
    b31 = p["mlp3_b1"].astype(np.float32)
    b32 = p["mlp3_b2"].astype(np.float32)
    bfco = p["fco_b"].astype(np.float32)

    wpackT = np.concatenate(
        [WpT, m1w1, m1w2, W21, m2w2, m3w1[0:H],
         m3w1[H:2 * H], m3w1[2 * H:3 * H], m3w2,
         np.eye(H, dtype=np.float32), p["fco_w"]],
        axis=1).astype(BF16)
    bpack = np.zeros((H, 9), np.float32)
    for k, v in enumerate([p["conv1_b"], b2p, b11, b12, b21, b22, b31, b32]):
        bpack[:, k] = v
    bpack[:O, 8] = bfco
    shared = {
        "p1": P1.reshape(B, CK, N * PT).astype(BF16),
        "w1pack": np.concatenate([W1s, W1r], axis=1).astype(BF16),
        "wpackF": np.concatenate(W2k + [wa], axis=1).astype(BF16),
        "wpackT": wpackT,
        "bpack": bpack,
        # 1/CT of the temporal mean folded in (attention weights are kept
        # unnormalized by CT on-device; V_all columns are CT x larger).
        "wph": (WpT / CT).astype(np.float16),
    }

    in_maps = []
    out_map = []  # (core, e_loc, e_glob)
    for c in range(N_CORES):
        recvs = list(range(c * RPC, (c + 1) * RPC))
        relr = np.zeros((EL, RPC), np.float32)
        relsT = np.zeros((N, EL), np.float32)
        relrT = np.zeros((N, EL), np.float32)
        for rr_i, r in enumerate(recvs):
            for s in range(N):
                if s == r:
                    continue
                e_loc = rr_i * N + s
                e_g = edge_of[(r, s)]
                relr[e_loc, rr_i] = 1.0
                relsT[s, e_loc] = 1.0
                relrT[rr_i, e_loc] = 1.0
                out_map.append((c, e_loc, e_g))
        m = dict(shared)
        m["p1r"] = np.ascontiguousarray(
            P1[:, :, recvs, :]).reshape(B, CK, RPC * PT).astype(BF16)
        m["rel_r"] = relr.astype(BF16)
        m["relT"] = np.concatenate([relsT, relrT], axis=1).astype(BF16)
        in_maps.append(m)
    return in_maps, out_map


def kernel(**inputs):
    rel_rec = np.asarray(inputs["rel_rec"])
    rel_send = np.asarray(inputs["rel_send"])
    x = np.asarray(inputs["inputs"])
    p = {k: np.asarray(v) for k, v in inputs.items()
         if k not in ("inputs", "rel_rec", "rel_send")}

    edge_of = _nri_structure(rel_rec, rel_send)
    if edge_of is None or x.shape != (B, N, T, D):
        # Inputs without the NRI one-hot structure: fall back to a plain
        # numpy evaluation (correctness path only).
        return _np_forward(x, rel_rec, rel_send, p).astype(np.float32)

    from concourse.bass_utils import run_bass_kernel_spmd

    nc = _build_program()
    in_maps, out_map = _host_prep(x, rel_rec, rel_send, p, edge_of)
    res = run_bass_kernel_spmd(nc, in_maps, list(range(N_CORES)),
                               trace=TRACE)
    if TRACE:
        global LAST_RESULT
        LAST_RESULT = res

    full = np.empty((B, E, O), np.float32)
    for c, e_loc, e_g in out_map:
        full[:, e_g, :] = res.results[c]["y"][:, :, e_loc]
    return full


# revision 55
# speedup vs baseline: 1.0962x; 1.0315x over previous
"""Trainium2 Bass kernel for the NRI CNNEncoder (gnn_message_passing).

Strategy
--------
8-way shard over the edge dimension E=1560: each core owns 5 receiver nodes
x 40 sender slots (the self-edge is computed as padding and discarded on the
host) = 200 local edges x B=4 batches = 800 edge-sequences per core.

Algebraic restructuring (all exact, eval-mode):
- conv1 is linear, so per-edge conv1(concat(send, recv)) = F_s[send] + F_r[recv]
  where F_s/F_r are convolutions of the 160 node sequences with the two halves
  of conv1_w: a 39x compute reduction on conv1.
- BatchNorm (eval) = per-channel positive-scale affine; it commutes with
  maxpool and folds into the following conv/matmul weights (bn1 -> conv2,
  bn2 -> convp/conva).
- convp (1x1) commutes with the attention-weighted temporal pooling, so it is
  applied after pooling: 44x less convp compute.
- ELU is stored shifted: elu(t)+1 = min(exp(t), 1+relu(t)) — two parallel
  engine ops + one combine; the "+1" is folded into the next layer's bias on
  the host.
- edge2node is local per receiver shard; the node2edge gather needs one
  per-batch AllGather of the mlp2 outputs (1.25KB). A dummy AllGather at
  kernel start absorbs the ~50us first-use collective setup.

Matmuls run in bf16 (fp32 PSUM accumulation); softmax in fp32.
"""

import os
import sys
import numpy as np

sys.path.insert(0, "/opt/trn_rl_repo")

import ml_dtypes

BF16 = ml_dtypes.bfloat16

# Problem constants (hardcoded; must match the reference).
B, N, T, D, H, O = 4, 40, 100, 4, 128, 2
E = N * (N - 1)          # 1560
BN_EPS = 1e-5
N_CORES = 8
RPC = N // N_CORES       # receivers per core = 5
EL = RPC * N             # local edges per core (incl. self padding) = 200
PT = T - 4               # conv1 output length = 96
PL = PT // 2             # pooled length = 48
CT = PL - 4              # conv2 output length = 44
CK = 20                  # conv1 contraction = D * K = 4*5
FSTRIP = 480             # F matmul strip (5 nodes x 96)
C2EDGES = 10             # edges per conv2/logits strip
C2STRIP = C2EDGES * CT   # 440


def _np_forward(inputs, rel_rec, rel_send, p):
    """Pure-numpy fp32 replica of the reference (fallback for inputs whose
    rel matrices do not have the NRI one-hot structure)."""
    x32 = inputs.astype(np.float32)
    rr = rel_rec.astype(np.float32)
    rs = rel_send.astype(np.float32)
    xf = x32.reshape(B, N, T * D)
    recv = np.einsum("en,bnf->bef", rr, xf).reshape(B * rr.shape[0], T, D)
    send = np.einsum("en,bnf->bef", rs, xf).reshape(B * rs.shape[0], T, D)
    x = np.concatenate([send.transpose(0, 2, 1), recv.transpose(0, 2, 1)], axis=1)

    def conv1d(x, w, b):
        k = w.shape[2]
        t_out = x.shape[2] - k + 1
        y = np.zeros((x.shape[0], w.shape[0], t_out), np.float32)
        for kk in range(k):
            y += np.einsum("oc,nct->not", w[:, :, kk], x[:, :, kk:kk + t_out])
        return y + b[None, :, None]

    def bn(x, g, b, m, v):
        return (x - m[None, :, None]) / np.sqrt(v[None, :, None] + BN_EPS) \
            * g[None, :, None] + b[None, :, None]

    def elu(x):
        return np.where(x > 0, x, np.expm1(x))

    def mlp(x, w1, b1, w2, b2):
        h = elu(x @ w1 + b1)
        return elu(h @ w2 + b2)

    x = bn(np.maximum(conv1d(x, p["conv1_w"], p["conv1_b"]), 0.0),
           p["bn1_g"], p["bn1_b"], p["bn1_m"], p["bn1_v"])
    n_, c_, t_ = x.shape
    x = x.reshape(n_, c_, t_ // 2, 2).max(axis=-1)
    x = bn(np.maximum(conv1d(x, p["conv2_w"], p["conv2_b"]), 0.0),
           p["bn2_g"], p["bn2_b"], p["bn2_m"], p["bn2_v"])
    pred = conv1d(x, p["convp_w"], p["convp_b"])
    a = conv1d(x, p["conva_w"], p["conva_b"])
    a = np.exp(a - a.max(axis=2, keepdims=True))
    a = a / a.sum(axis=2, keepdims=True)
    x = (pred * a).mean(axis=2).reshape(B, -1, H)
    x = mlp(x, p["mlp1_w1"], p["mlp1_b1"], p["mlp1_w2"], p["mlp1_b2"])
    x_skip = x
    inc = np.einsum("en,beh->bnh", rr, x) / N
    x = mlp(inc, p["mlp2_w1"], p["mlp2_b1"], p["mlp2_w2"], p["mlp2_b2"])
    sn = np.einsum("en,bnh->beh", rs, x)
    rc = np.einsum("en,bnh->beh", rr, x)
    x = np.concatenate([sn, rc, x_skip], axis=2)
    x = mlp(x, p["mlp3_w1"], p["mlp3_b1"], p["mlp3_w2"], p["mlp3_b2"])
    return x @ p["fco_w"] + p["fco_b"]


def _nri_structure(rel_rec, rel_send):
    """If (rel_rec, rel_send) are the NRI fully-connected one-hot matrices,
    return edge_of[r][s] -> global edge index; else None."""
    if rel_rec.shape != (E, N) or rel_send.shape != (E, N):
        return None
    rec_i = np.argmax(rel_rec, axis=1)
    snd_i = np.argmax(rel_send, axis=1)
    eye = np.eye(N, dtype=rel_rec.dtype)
    if not (np.array_equal(rel_rec, eye[rec_i]) and
            np.array_equal(rel_send, eye[snd_i])):
        return None
    edge_of = {}
    for e in range(E):
        r, s = int(rec_i[e]), int(snd_i[e])
        if r == s or (r, s) in edge_of:
            return None
        edge_of[(r, s)] = e
    if len(edge_of) != E:
        return None
    return edge_of


_PROGRAM_CACHE = {}
TRACE = False          # test harness sets True to collect NTFF exec time
LAST_RESULT = None     # BassKernelResults of the last run (when TRACE)


def _build_program():
    """Build + compile the SPMD Bass program (cached per process)."""
    if "nc" in _PROGRAM_CACHE:
        return _PROGRAM_CACHE["nc"]

    import concourse.bacc as bacc
    import concourse.tile as tile
    from concourse import mybir
    from contextlib import ExitStack

    f32 = mybir.dt.float32
    f16 = mybir.dt.float16
    bf16 = mybir.dt.bfloat16
    Alu = mybir.AluOpType
    Act = mybir.ActivationFunctionType

    nc = bacc.Bacc("TRN2", target_bir_lowering=False, debug=False,
                   num_devices=N_CORES)

    def din(name, shape, dt=bf16):
        return nc.dram_tensor(name, shape, dt, kind="ExternalInput").ap()

    p1 = din("p1", [B, CK, N * PT])
    p1r = din("p1r", [B, CK, RPC * PT])
    rel_r = din("rel_r", [EL, RPC])
    w1pack = din("w1pack", [CK, 2 * H])
    wpackF = din("wpackF", [H, 5 * H + 1])
    wpackT = din("wpackT", [H, 10 * H + O])
    bpack = din("bpack", [H, 9], f32)
    relT = din("relT", [N, 2 * EL])
    wph = din("wph", [H, H], f16)

    y = nc.dram_tensor("y", [B, O, EL], f32, kind="ExternalOutput").ap()
    # AllGather payload: per-core [RPC, H] bf16 (only q_a = w31a^T @ x2 is
    # gathered; the receiver term stays local).
    cc_in = nc.dram_tensor("cc_in", [B, RPC, H], bf16).ap()
    cc_out = nc.dram_tensor("cc_out", [B, N, H], bf16,
                        addr_space="Shared").ap()
    # Dummy collective to absorb the ~50us first-use setup cost.
    wu_in = nc.dram_tensor("wu_in", [1, 4], bf16).ap()
    wu_out = nc.dram_tensor("wu_out", [N_CORES, 4], bf16,
                            addr_space="Shared").ap()

    with tile.TileContext(nc) as tc:
        with ExitStack() as ctx:
            singles = ctx.enter_context(tc.tile_pool(name="singles", bufs=1))
            work = ctx.enter_context(tc.tile_pool(name="work", bufs=2))
            work3 = ctx.enter_context(tc.tile_pool(name="work3", bufs=3))
            psum = ctx.enter_context(
                tc.tile_pool(name="psum", bufs=2, space="PSUM"))
            psumc = ctx.enter_context(
                tc.tile_pool(name="psumc", bufs=2, space="PSUM"))
            psuml = ctx.enter_context(
                tc.tile_pool(name="psuml", bufs=1, space="PSUM"))
            dpool = ctx.enter_context(
                tc.tile_pool(name="dpool", bufs=2, space="DRAM"))

            def sload(ap_dram, shape, dt=bf16, name=None, eng=None):
                t = singles.tile(shape, dt,
                                 name=name or f"c_{ap_dram.tensor.name}")
                (eng or nc.sync).dma_start(out=t[:], in_=ap_dram)
                return t

            # --- collective warm-up (first thing on the CC queue) --------
            wu_sb = singles.tile([1, 4], bf16, name="wu_sb")
            nc.vector.memset(wu_sb[:], 0.0)
            nc.gpsimd.dma_start(out=wu_in, in_=wu_sb[:])
            nc.gpsimd.collective_compute(
                "AllGather", mybir.AluOpType.bypass,
                replica_groups=[list(range(N_CORES))],
                ins=[wu_in], outs=[wu_out])

            # --- weights / constants into SBUF (packed DMAs) ------------
            # w1pack + batch 0's im2col inputs first: the first f_stage
            # needs exactly these, everything else can trickle in after.
            w1pack_sb = sload(w1pack, [CK, 2 * H])
            p1s0 = work.tile([CK, N * PT], bf16, tag="p1_sb",
                             name="p1_sb", bufs=4)
            nc.sync.dma_start(out=p1s0[:], in_=p1[0])
            p1r_s0 = work.tile([CK, RPC * PT], bf16, tag="p1r_sb",
                               name="p1r_sb", bufs=4)
            nc.sync.dma_start(out=p1r_s0[:], in_=p1r[0])
            w1s_sb = w1pack_sb[:, 0:H]
            w1r_sb = w1pack_sb[:, H:2 * H]
            wpackF_sb = sload(wpackF, [H, 5 * H + 1])
            w2_sb = [wpackF_sb[:, k * H:(k + 1) * H] for k in range(5)]
            wa_sb = wpackF_sb[:, 5 * H:5 * H + 1]
            wpackT_sb = sload(wpackT, [H, 10 * H + O], eng=nc.scalar)
            (wp_sb, w11_sb, w12_sb, w21_sb, w22_sb, w31a_sb, w31b_sb,
             w31c_sb, w32_sb, ident_sb) = [
                wpackT_sb[:, k * H:(k + 1) * H] for k in range(10)]
            w31ab_sb = wpackT_sb[:, 5 * H:7 * H]
            wfco_sb = wpackT_sb[:, 10 * H:10 * H + O]
            bpack_sb = sload(bpack, [H, 9], f32)
            (b1_sb, b2p_sb, b11_sb, b12_sb, b21_sb, b22_sb, b31_sb,
             b32_sb) = [bpack_sb[:, k:k + 1] for k in range(8)]
            bfco_sb = bpack_sb[0:O, 8:9]
            rel_ra_sb = sload(rel_r[0:120, :], [120, RPC], name="rel_ra",
                              eng=nc.gpsimd)
            rel_rb_sb = sload(rel_r[120:EL, :], [EL - 120, RPC],
                              name="rel_rb", eng=nc.gpsimd)
            relT_sb = sload(relT, [N, 2 * EL], eng=nc.gpsimd)
            rel_sT_sb = relT_sb[:, 0:EL]
            rel_rT_sb = relT_sb[0:RPC, EL:2 * EL]

            wph_sb = sload(wph, [H, H], dt=f16, name="c_wph")

            # --- persistent accumulators ------------------------------------
            V_all = singles.tile([H, B * EL], f16, tag="V_all")
            X1T = singles.tile([H, B * EL], bf16, tag="X1T")

            def elu(ps, bias_sb, out_sb, comb=None, relu_eng=None):
                """out_sb(bf16) = elu(ps + bias) = min(exp(t)-1, relu(t)).
                Exact: for t<=0 exp(t)-1 = elu <= 0 = relu; for t>0
                exp(t)-1 >= t = relu. Overflow-safe (inf loses the min).
                exp and relu are independent -> run on parallel engines."""
                cols = ps.shape[1]
                ex = work.tile([ps.shape[0], cols], f32, tag="elu_ex")
                nc.scalar.activation(ex[:], ps[:], Act.Exp, bias=bias_sb[:])
                rl = work.tile([ps.shape[0], cols], f32, tag="elu_rl")
                if relu_eng is None:
                    nc.scalar.activation(rl[:], ps[:], Act.Relu,
                                         bias=bias_sb[:])
                else:
                    relu_eng.tensor_scalar(
                        out=rl[:], in0=ps[:], scalar1=bias_sb[:],
                        scalar2=0.0, op0=Alu.add, op1=Alu.max)
                (comb or nc.vector).scalar_tensor_tensor(
                    out=out_sb, in0=ex[:], scalar=-1.0, in1=rl[:],
                    op0=Alu.add, op1=Alu.min)

            # ================= per-batch edge pipeline =======================
            def a_front_dve(b, rr, Fs, Fr):
                """G and fused pool+relu (the DVE half of the block front)."""
                G = work.tile([H, N * PT], bf16, tag="G", name="G")
                fr_b = Fr[:, rr * PT:(rr + 1) * PT] \
                    .unsqueeze(1).broadcast_to([H, N, PT])
                nc.vector.tensor_tensor(
                    out=G[:].rearrange("p (n t) -> p n t", t=PT),
                    in0=Fs[:].rearrange("p (n t) -> p n t", t=PT),
                    in1=fr_b, op=Alu.add)
                # fused maxpool(k=2) + relu on DVE: max(G_even, G_odd, 0).
                # The host de-interleaves conv1's time columns (even block
                # then odd block per node), so both pool inputs and the
                # output are packed contiguous runs -> DVE 2x mode.
                Y1 = work.tile([H, N * PL], bf16, tag="Y1", name="Y1")
                G3 = G[:].rearrange("p (n x) -> p n x", x=PT)
                nc.vector.scalar_tensor_tensor(
                    out=Y1[:].rearrange("p (e t) -> p e t", t=PL),
                    in0=G3[:, :, 0:PL],
                    scalar=0.0, in1=G3[:, :, PL:PT],
                    op0=Alu.max, op1=Alu.max)
                return Y1

            def a_front_pe(Y1):
                """conv2 matmuls (queued on PE after the small phase-b
                matmuls so those aren't stuck behind the conv2 burst)."""
                Y1r = Y1[:].rearrange("p (e t) -> p e t", t=PL)
                # conv2 into two 2-bank PSUM tiles (strips padded to 512
                # cols) so the relu drain needs 2 instructions, not 4.
                # Tap-outer within each PSUM tile: each weight tile loads
                # twice per block (10 LDWEIGHTS instead of 20) while the
                # first tile still completes halfway through the block.
                c2ps = []
                for h in range(2):
                    ps = psumc.tile([H, 1024], f32, tag="c2",
                                    name=f"c2ps{h}")
                    for k in range(5):
                        for sl in range(2):
                            st = 2 * h + sl
                            nc.tensor.matmul(
                                ps[:, sl * 512:sl * 512 + C2STRIP],
                                lhsT=w2_sb[k][:],
                                rhs=Y1r[:, st * C2EDGES:(st + 1) * C2EDGES,
                                        k:k + CT],
                                start=(k == 0), stop=(k == 4))
                    c2ps.append(ps)
                return (c2ps,)

            def a_tail(b, rr, c2ps):
                """relu2 (PSUM drain), logits, direct PSUM->A_t DMAs."""
                Y = work.tile([H, N * CT], bf16, tag="Y", name="Y", bufs=6)
                for h in range(2):
                    nc.scalar.activation(
                        Y[:, h * 2 * C2STRIP:(h + 1) * 2 * C2STRIP]
                        .rearrange("p (s x) -> p s x", x=C2STRIP),
                        c2ps[h][:].rearrange("p (s x) -> p s x",
                                             x=512)[:, :, 0:C2STRIP],
                        Act.Relu, bias=b2p_sb[:])
                A_t = work.tile([N, CT], f32, tag="A_t", name="A_t", bufs=4)
                Lsb = work.tile([1, N * CT], f32, tag="Lsb", name="Lsb",
                                bufs=3)
                # Two logit strips per 2-bank PSUM tile; each pair drained
                # by one copy (alternating Scalar/Pool to balance queues).
                for h2 in range(2):
                    lps = psuml.tile([1, 1024], f32, tag="lp", name="lps")
                    for sl in range(2):
                        st = 2 * h2 + sl
                        nc.tensor.matmul(
                            lps[:, sl * 512:sl * 512 + C2STRIP],
                            lhsT=wa_sb[:],
                            rhs=Y[:, st * C2STRIP:(st + 1) * C2STRIP],
                            start=True, stop=True)
                    nc.scalar.copy(
                        Lsb[:, h2 * 2 * C2STRIP:(h2 + 1) * 2 * C2STRIP]
                        .rearrange("p (s x) -> p s x", x=C2STRIP),
                        lps[:].rearrange("p (s x) -> p s x",
                                         x=512)[:, :, 0:C2STRIP])
                nc.sync.dma_start(out=A_t[:], in_=Lsb[:])
                return Y, A_t

            def sm_stage(b, rr, Y, A_t):
                """Softmax + partition-broadcast. No max-subtraction: the
                attention logits here are O(1), far from fp32 exp range."""
                Ex = work.tile([N, CT], f32, tag="Ex", name="Ex")
                S = work.tile([N, 1], f32, tag="S", name="S")
                nc.scalar.activation(Ex[:], A_t[:], Act.Exp,
                                     accum_out=S[:])
                rz = work.tile([N, 1], f32, tag="rz", name="rz")
                nc.vector.reciprocal(rz[:], S[:])
                # Normalize on Scalar (copy with per-partition scale); the
                # 1/CT of the temporal mean is folded into wph on the host.
                A_bf = work.tile([N, CT], bf16, tag="A_bf", name="A_bf")
                nc.scalar.activation(A_bf[:], Ex[:], Act.Copy,
                                     scale=rz[:])
                # Bounce through DRAM to broadcast across partitions.
                A_dram = dpool.tile([1, N * CT], bf16, tag="A_dram",
                                    name="A_dram", bufs=5)
                nc.gpsimd.dma_start(out=A_dram[:], in_=A_bf[:])
                A_bc = work.tile([H, N * CT], bf16, tag="A_bc", name="A_bc",
                                 bufs=6)
                # On the Pool queue (right after the A_dram write) so its
                # dependency wait never head-of-line-blocks the A_t DMAs,
                # which stay alone on the Sync queue.
                nc.gpsimd.dma_start(
                    out=A_bc[:],
                    in_=A_dram[0:1, :].broadcast_to([H, N * CT]))
                return b, rr, Y, A_bc

            def mt_mult(b, rr, Y, A_bc):
                """Weighted temporal multiply. Runs on the Pool engine for
                some blocks to relieve the DVE; the segmented reduce is
                issued one iteration later (mt_reduce) so the DVE FIFO
                never stalls waiting for the slower Pool multiply."""
                eng = nc.gpsimd if rr in (1, 3) else nc.vector
                Mt = work.tile([H, N * CT], f16, tag="Mt", name="Mt",
                               bufs=3)
                eng.tensor_tensor(out=Mt[:], in0=Y[:], in1=A_bc[:],
                                  op=Alu.mult)
                return b, rr, Mt

            def mt_reduce(b, rr, Mt):
                col0 = (b * RPC + rr) * N
                with nc.allow_low_precision(
                        reason="fp16 attention-pool accum, |terms|<=44"):
                    nc.vector.tensor_reduce(
                        out=V_all[:, col0:col0 + N],
                        in_=Mt[:].rearrange("p (e t) -> p e t", t=CT),
                        axis=mybir.AxisListType.X, op=Alu.add)

            def phase_b_part(b, c0, cw, comb=None):
                """convp (folded post-pooling) + mlp1 for cols [c0, c0+cw)
                of batch b."""
                cs = slice(b * EL + c0, b * EL + c0 + cw)
                zps = psum.tile([H, cw], f32, tag="ps", name="zps")
                nc.tensor.matmul(zps[:], lhsT=wph_sb[:], rhs=V_all[:, cs],
                                 start=True, stop=True)
                xsb = work.tile([H, cw], bf16, tag="xsb", name="xsb")
                nc.scalar.copy(xsb[:], zps[:])
                h1ps = psum.tile([H, cw], f32, tag="ps", name="h1ps")
                nc.tensor.matmul(h1ps[:], lhsT=w11_sb[:], rhs=xsb[:],
                                 start=True, stop=True)
                h1sb = work.tile([H, cw], bf16, tag="h1sb", name="h1sb")
                elu(h1ps, b11_sb, h1sb[:], comb=comb)
                h2ps = psum.tile([H, cw], f32, tag="ps", name="h2ps")
                nc.tensor.matmul(h2ps[:], lhsT=w12_sb[:], rhs=h1sb[:],
                                 start=True, stop=True)
                elu(h2ps, b12_sb, X1T[:, cs], comb=comb)

            h3part = singles.tile([H, 2 * EL], bf16, tag="h3part")

            INC_CHUNKS = [(0, 120), (120, EL - 120)]

            def inc_tr(b, j, incps):
                """Transpose one X1T chunk of batch b and accumulate its
                edge2node partial into incps."""
                c0, cw = INC_CHUNKS[j]
                tps = psum.tile([cw, H], bf16, tag="ps", name="tps")
                nc.tensor.transpose(
                    tps[:], in_=X1T[:, b * EL + c0:b * EL + c0 + cw],
                    identity=ident_sb[:])
                x1e = work3.tile([cw, H], bf16, tag=f"x1e{j}", name="x1e")
                nc.scalar.copy(x1e[:], tps[:])
                rel_chunk = rel_ra_sb if j == 0 else rel_rb_sb
                nc.tensor.matmul(
                    incps[:], lhsT=x1e[:],
                    rhs=rel_chunk[:], start=(j == 0), stop=(j == 1))

            def inc_fin(b, incps):
                """mlp2 on the local RPC nodes + mlp3 weight folding ->
                [RPC, H] gather payload."""
                inc_sb = work3.tile([H, RPC], bf16, tag="inc_sb",
                                    name="inc_sb")
                nc.scalar.copy(inc_sb[:], incps[:])
                m2ps = psum.tile([H, RPC], f32, tag="ps", name="m2ps")
                nc.tensor.matmul(m2ps[:], lhsT=w21_sb[:], rhs=inc_sb[:],
                                 start=True, stop=True)
                m2sb = work3.tile([H, RPC], bf16, tag="m2sb", name="m2sb")
                elu(m2ps, b21_sb, m2sb[:], relu_eng=nc.vector)
                m2ps2 = psum.tile([H, RPC], f32, tag="ps", name="m2ps2")
                nc.tensor.matmul(m2ps2[:], lhsT=w22_sb[:], rhs=m2sb[:],
                                 start=True, stop=True)
                x2t = work3.tile([H, RPC], bf16, tag="x2t", name="x2t")
                elu(m2ps2, b22_sb, x2t[:], relu_eng=nc.vector)
                # qaT = x2^T @ w31a (sender term -> gathered),
                # qbT = x2^T @ w31b (receiver term -> stays local);
                # one merged matmul [RPC, 2H]; lhsT = x2 keeps the gather
                # payload and the rc-gather lhsT at partition 0.
                qps = psum.tile([RPC, 2 * H], f32, tag="ps", name="qps")
                nc.tensor.matmul(qps[:], lhsT=x2t[:], rhs=w31ab_sb[:],
                                 start=True, stop=True)
                qt_sb = work3.tile([RPC, 2 * H], bf16, tag="qt_sb",
                                   name="qt_sb")
                nc.scalar.copy(qt_sb[:], qps[:])
                nc.gpsimd.dma_start(out=cc_in[b], in_=qt_sb[:, 0:H])
                return qt_sb

            def cc_b(b):
                nc.gpsimd.collective_compute(
                    "AllGather", mybir.AluOpType.bypass,
                    replica_groups=[list(range(N_CORES))],
                    ins=[cc_in[b]], outs=[cc_out[b]])

            def h3_pre(b, qt_sb):
                """Pre-collective part of mlp3: receiver + skip terms."""
                h3p = psum.tile([H, EL], f32, tag="ps", name="h3p")
                nc.tensor.matmul(h3p[:], lhsT=qt_sb[:, H:2 * H],
                                 rhs=rel_rT_sb[:], start=True, stop=False)
                nc.tensor.matmul(h3p[:], lhsT=w31c_sb[:],
                                 rhs=X1T[:, b * EL:(b + 1) * EL],
                                 start=False, stop=True)
                nc.scalar.copy(h3part[:, (b % 2) * EL:(b % 2 + 1) * EL],
                               h3p[:])

            def f_dma(b):
                """Prefetch the im2col inputs for batch b. All four batches
                are issued upfront on the gpsimd queue (cheap issue, no
                data-dependent traffic ahead of them)."""
                p1_sb = work.tile([CK, N * PT], bf16, tag="p1_sb",
                                  name="p1_sb", bufs=4)
                nc.sync.dma_start(out=p1_sb[:], in_=p1[b])
                p1r_sb = work.tile([CK, RPC * PT], bf16, tag="p1r_sb",
                                   name="p1r_sb", bufs=4)
                nc.sync.dma_start(out=p1r_sb[:], in_=p1r[b])
                return p1_sb, p1r_sb

            def f_stage(b, p1_sb, p1r_sb):
                """conv1 of the node sequences. Uses single-strip tiles
                from the small "ps" pool so it never contends with conv2's
                psumc buffers (which caused batch-boundary stalls)."""
                Fs = work.tile([H, N * PT], bf16, tag="Fs", name="Fs")
                for s8 in range(N * PT // FSTRIP):
                    fps = psum.tile([H, FSTRIP], f32, tag="ps", name="fps")
                    nc.tensor.matmul(
                        fps[:], lhsT=w1s_sb[:],
                        rhs=p1_sb[:, s8 * FSTRIP:(s8 + 1) * FSTRIP],
                        start=True, stop=True)
                    nc.scalar.copy(
                        Fs[:, s8 * FSTRIP:(s8 + 1) * FSTRIP], fps[:])
                Fr = work.tile([H, RPC * PT], bf16, tag="Fr", name="Fr")
                frps = psum.tile([H, RPC * PT], f32, tag="ps", name="frps")
                nc.tensor.matmul(frps[:], lhsT=w1r_sb[:], rhs=p1r_sb[:],
                                 start=True, stop=True)
                nc.scalar.activation(Fr[:], frps[:], Act.Identity,
                                     bias=b1_sb[:])
                return Fs, Fr

            def tail_stage(b):
                """Post-collective: sender gather + mlp3 + fco + output."""
                qg_sb = work.tile([N, H], bf16, tag="qg", name="qg_sb")
                nc.sync.dma_start(out=qg_sb[:], in_=cc_out[b])
                h3ps = psum.tile([H, EL], f32, tag="ps", name="h3ps")
                nc.tensor.matmul(h3ps[:], lhsT=qg_sb[:], rhs=rel_sT_sb[:],
                                 start=True, stop=False)
                nc.tensor.matmul(h3ps[:], lhsT=ident_sb[:],
                                 rhs=h3part[:, (b % 2) * EL:(b % 2 + 1) * EL],
                                 start=False, stop=True)
                h3sb = work.tile([H, EL], bf16, tag="h3sb", name="h3sb")
                elu(h3ps, b31_sb, h3sb[:], relu_eng=nc.vector)
                h4ps = psum.tile([H, EL], f32, tag="ps", name="h4ps")
                nc.tensor.matmul(h4ps[:], lhsT=w32_sb[:], rhs=h3sb[:],
                                 start=True, stop=True)
                h4sb = work.tile([H, EL], bf16, tag="h4sb", name="h4sb")
                elu(h4ps, b32_sb, h4sb[:], relu_eng=nc.vector)
                ops = psum.tile([O, EL], f32, tag="ps", name="ops")
                nc.tensor.matmul(ops[:], lhsT=wfco_sb[:], rhs=h4sb[:],
                                 start=True, stop=True)
                osb = work.tile([O, EL], f32, tag="osb", name="osb")
                nc.vector.tensor_scalar_add(osb[:], ops[:], bfco_sb[:])
                nc.sync.dma_start(out=y[b], in_=osb[:])

            # Software pipeline over edge-blocks:
            #   a_front(i) -> sm(i-1) -> mt(i-2)+convp/mlp1 chunk -> a_tail(i)
            # with per-batch edge2node/mlp2/AllGather and the post-collective
            # tail overlapped under later blocks.
            # Software pipeline (block i processed at iteration i):
            #   a_tail(i) -> sm(i) at iter i+2 -> Mt multiply at iter i+4
            #   -> segmented reduce + convp/mlp1 chunk at iter i+5.
            # Every cross-engine hop gets at least a full iteration of
            # slack so the strictly-FIFO engine queues never stall on a
            # not-yet-ready dependency; the reduce lags the multiply so
            # the DVE never waits on the Pool engine.
            from collections import deque
            pend_sms = deque()
            pend_mults = deque()
            pend_red = None
            FsFr = None
            qts = {}
            p1s = [(p1s0, p1r_s0)] + [f_dma(b) for b in range(1, B)]
            blocks = [(b, rr) for b in range(B) for rr in range(RPC)]

            def do_reduce_and_part(red):
                rb, rrr, Mt = red
                mt_reduce(rb, rrr, Mt)
                phase_b_part(rb, rrr * N, N)

            def batch_tail(bb):
                """Strip-level edge2node -> mlp2 -> AllGather issue."""
                incps = psum.tile([H, RPC], f32, tag="ps", name="incps")
                inc_tr(bb, 0, incps)
                inc_tr(bb, 1, incps)
                qts[bb] = inc_fin(bb, incps)
                cc_b(bb)

            FsFr_next = None
            for i, (b, rr) in enumerate(blocks):
                if i == 0:
                    FsFr = f_stage(0, *p1s[0])
                elif rr == 0:
                    FsFr = FsFr_next
                Y1 = a_front_dve(b, rr, *FsFr)
                if pend_red is not None:
                    do_reduce_and_part(pend_red)
                    pend_red = None
                fr = a_front_pe(Y1)
                if len(pend_mults) >= 2:
                    pend_red = mt_mult(*pend_mults.popleft())
                if len(pend_sms) >= 2:
                    pend_mults.append(sm_stage(*pend_sms.popleft()))
                tiles = a_tail(b, rr, *fr)
                pend_sms.append((b, rr) + tiles)
                if rr == 3 and b + 1 < B:
                    # conv1 for the next batch, two blocks early: its
                    # matmuls/drains hide under this batch's pipeline.
                    FsFr_next = f_stage(b + 1, *p1s[b + 1])
                if i in (9, 14, 19):
                    batch_tail((i - 9) // 5)
                if i in (10, 15):
                    h3_pre((i - 10) // 5, qts[(i - 10) // 5])
                if i in (12, 17):
                    tail_stage((i - 12) // 5)
            # Endgame: drain the 5-stage pipeline for batch 3, then its
            # serial tail: edge2node -> mlp2 -> AllGather -> mlp3 -> out.
            do_reduce_and_part(pend_red)                         # (3,0)
            m = mt_mult(*pend_mults.popleft())                   # (3,1)
            pend_mults.append(sm_stage(*pend_sms.popleft()))     # sm(3,3)
            do_reduce_and_part(m)
            m = mt_mult(*pend_mults.popleft())                   # (3,2)
            pend_mults.append(sm_stage(*pend_sms.popleft()))     # sm(3,4)
            h3_pre(2, qts[2])
            do_reduce_and_part(m)
            m = mt_mult(*pend_mults.popleft())                   # (3,3)
            do_reduce_and_part(m)
            m = mt_mult(*pend_mults.popleft())                   # (3,4)
            tail_stage(2)
            do_reduce_and_part(m)
            incps3 = psum.tile([H, RPC], f32, tag="ps", name="incps")
            inc_tr(3, 0, incps3)
            inc_tr(3, 1, incps3)
            qt = inc_fin(3, incps3)
            cc_b(3)
            h3_pre(3, qt)
            tail_stage(3)

    nc.compile()
    _PROGRAM_CACHE["nc"] = nc
    return nc


def _host_prep(inputs, rel_rec, rel_send, p, edge_of):
    """Build the per-core input maps + (core, local, global) output mapping."""
    x = inputs.astype(np.float32)
    # im2col of the node time-series: P1[b, c*5+k, n*96+t] = x[b, n, t+k, c]
    win = np.lib.stride_tricks.sliding_window_view(x, 5, axis=2)  # [B,N,96,D,5]
    P1 = win.transpose(0, 3, 4, 1, 2).reshape(B, CK, N, PT)
    # De-interleave each node's conv1 time columns (even positions first,
    # then odd) so the kernel's fused maxpool reads packed contiguous runs
    # (DVE 2x mode) instead of stride-2 views.
    perm = np.concatenate([np.arange(0, PT, 2), np.arange(1, PT, 2)])
    P1 = np.ascontiguousarray(P1[:, :, :, perm])

    a1 = (p["bn1_g"] / np.sqrt(p["bn1_v"] + BN_EPS)).astype(np.float32)
    c1 = (p["bn1_b"] - p["bn1_m"] * a1).astype(np.float32)
    a2 = (p["bn2_g"] / np.sqrt(p["bn2_v"] + BN_EPS)).astype(np.float32)
    c2 = (p["bn2_b"] - p["bn2_m"] * a2).astype(np.float32)

    w1 = p["conv1_w"].astype(np.float32)           # [H, 2D, 5]
    # rows ordered c*5+k to match P1
    W1s = w1[:, :D, :].transpose(1, 2, 0).reshape(CK, H)
    W1r = w1[:, D:, :].transpose(1, 2, 0).reshape(CK, H)

    w2f = p["conv2_w"].astype(np.float32) * a1[None, :, None]   # [o,i,k]
    b2p = p["conv2_b"].astype(np.float32) + np.einsum(
        "oik,i->o", p["conv2_w"].astype(np.float32), c1)
    W2k = [w2f[:, :, k].T.copy() for k in range(5)]             # lhsT [i,o]

    wa = (p["conva_w"][0, :, 0].astype(np.float32) * a2)[:, None]  # [H,1]
    WpT = (p["convp_w"][:, :, 0].astype(np.float32) * a2[None, :]).T  # [i,o]
    bpp = p["convp_b"].astype(np.float32) + \
        p["convp_w"][:, :, 0].astype(np.float32) @ c2

    m1w1 = p["mlp1_w1"].astype(np.float32)
    m1w2 = p["mlp1_w2"].astype(np.float32)
    b11 = p["mlp1_b1"].astype(np.float32) + (bpp / CT) @ m1w1
    b12 = p["mlp1_b2"].astype(np.float32)
    m2w1 = p["mlp2_w1"].astype(np.float32)
    m2w2 = p["mlp2_w2"].astype(np.float32)
    W21 = m2w1 / N
    b21 = p["mlp2_b1"].astype(np.float32)
    b22 = p["mlp2_b2"].astype(np.float32)
    m3w1 = p["mlp3_w1"].astype(np.float32)
    m3w2 = p["mlp3_w2"].astype(np.float32)
    b31 = p["mlp3_b1"].astype(np.float32)
    b32 = p["mlp3_b2"].astype(np.float32)
    bfco = p["fco_b"].astype(np.float32)

    wpackT = np.concatenate(
        [WpT, m1w1, m1w2, W21, m2w2, m3w1[0:H],
         m3w1[H:2 * H], m3w1[2 * H:3 * H], m3w2,
         np.eye(H, dtype=np.float32), p["fco_w"]],
        axis=1).astype(BF16)
    bpack = np.zeros((H, 9), np.float32)
    for k, v in enumerate([p["conv1_b"], b2p, b11, b12, b21, b22, b31, b32]):
        bpack[:, k] = v
    bpack[:O, 8] = bfco
    shared = {
        "p1": P1.reshape(B, CK, N * PT).astype(BF16),
        "w1pack": np.concatenate([W1s, W1r], axis=1).astype(BF16),
        "wpackF": np.concatenate(W2k + [wa], axis=1).astype(BF16),
        "wpackT": wpackT,
        "bpack": bpack,
        # 1/CT of the temporal mean folded in (attention weights are kept
        # unnormalized by CT on-device; V_all columns are CT x larger).
        "wph": (WpT / CT).astype(np.float16),
    }

    in_maps = []
    out_map = []  # (core, e_loc, e_glob)
    for c in range(N_CORES):
        recvs = list(range(c * RPC, (c + 1) * RPC))
        relr = np.zeros((EL, RPC), np.float32)
        relsT = np.zeros((N, EL), np.float32)
        relrT = np.zeros((N, EL), np.float32)
        for rr_i, r in enumerate(recvs):
            for s in range(N):
                if s == r:
                    continue
                e_loc = rr_i * N + s
                e_g = edge_of[(r, s)]
                relr[e_loc, rr_i] = 1.0
                relsT[s, e_loc] = 1.0
                relrT[rr_i, e_loc] = 1.0
                out_map.append((c, e_loc, e_g))
        m = dict(shared)
        m["p1r"] = np.ascontiguousarray(
            P1[:, :, recvs, :]).reshape(B, CK, RPC * PT).astype(BF16)
        m["rel_r"] = relr.astype(BF16)
        m["relT"] = np.concatenate([relsT, relrT], axis=1).astype(BF16)
        in_maps.append(m)
    return in_maps, out_map


def kernel(**inputs):
    rel_rec = np.asarray(inputs["rel_rec"])
    rel_send = np.asarray(inputs["rel_send"])
    x = np.asarray(inputs["inputs"])
    p = {k: np.asarray(v) for k, v in inputs.items()
         if k not in ("inputs", "rel_rec", "rel_send")}

    edge_of = _nri_structure(rel_rec, rel_send)
    if edge_of is None or x.shape != (B, N, T, D):
        # Inputs without the NRI one-hot structure: fall back to a plain
        # numpy evaluation (correctness path only).
        return _np_forward(x, rel_rec, rel_send, p).astype(np.float32)

    from concourse.bass_utils import run_bass_kernel_spmd

    nc = _build_program()
    in_maps, out_map = _host_prep(x, rel_rec, rel_send, p, edge_of)
    res = run_bass_kernel_spmd(nc, in_maps, list(range(N_CORES)),
                               trace=TRACE)
    if TRACE:
        global LAST_RESULT
        LAST_RESULT = res

    full = np.empty((B, E, O), np.float32)
    for c, e_loc, e_g in out_map:
        full[:, e_g, :] = res.results[c]["y"][:, :, e_loc]
    return full
